# revision 16
# baseline (speedup 1.0000x reference)
"""2-layer GCN on 8 Trainium2 NeuronCores (Bass/Tile SPMD kernel).

Math: reference computes, per layer,
    out = A_norm @ (in @ W) + b,   A_norm[d,s] = dis[d]*dis[s]*A_hat[d,s]
with A_hat = adjacency + self-loops, dis = 1/sqrt(deg).
We use associativity to aggregate first:
    out = dis ⊙ (A_hat @ (dis ⊙ in)) @ W + b
so the per-edge work is a pure gather+segment-sum of pre-scaled node
features (no per-edge multiplies).

Sharding: nodes are balanced across 8 cores x NSC superchunks of 512
"slots" each (NSC padded to a multiple of NB so AllGather quarters are
superchunk-aligned).  Edges are assigned to the core/superchunk of their
dst node, bucketed by src block (so gather indices fit in int16), sorted
by dst slot, and packed into 128-edge groups.  Each group is one one-hot
matmul G.T @ S that segment-sums the group into PSUM[feat, 512 slots].

Layer 1 consumes a HOST-pre-gathered fp8 edge-feature stream (xg) plus a
HOST-prebuilt fp8 one-hot stream (S) — both pure sequential DMA, no
on-chip gather or S-build.  Layer 2 gathers the fp16 hidden features
with pipelined dma_gather (prepare_only + per-queue trigger/sems) from
the AllGathered hidden state; its one-hot S is built on the DVE
(is_equal against an iota table) since the gathered operand is fp16.

Per-node output rows are stored "staged" (row = p*4 + j for slot
j*128+p) so each superchunk's four activation tiles land in one
contiguous [128, 4, F] DMA.

The gather-source row space is quartered: row = q*BLOCK + core*QR + o
so that each of 4 chunked AllGathers of the hidden layer delivers
exactly one gather bucket, letting layer-2 aggregation start while
later chunks are still in flight.
"""

import os
import sys

import ml_dtypes
import numpy as np

sys.path.insert(0, "/opt/trn_rl_repo")

P = 128          # partitions / group size
SC = 512         # slots per superchunk (= one PSUM bank of f32)
NCORES = 8
F_IN = 128
F_HID = 128
F_OUT = 64
NB = 4           # src buckets == AllGather chunks (int16 gather idx limit)
TPS = SC // P    # tiles per superchunk (4)

WMAX = int(os.environ.get("GCN2_WMAX", "64"))   # max slot span of a group
GCHUNK = int(os.environ.get("GCN2_L2G", "8"))  # ucode cap: 1024 idxs/call  # groups per gather call
XF8 = os.environ.get("GCN2_XF8", "1") == "1"    # fp8 layer-1 streams

F8 = ml_dtypes.float8_e4m3


def _chunks(ng):
    n = -(-ng // GCHUNK)
    base = ng // n
    rem = ng % n
    return [base + (1 if i < rem else 0) for i in range(n)]


# ----------------------------------------------------------------- host prep
def _prep(x, edge_index):
    N, F = x.shape
    assert F == F_IN
    src0 = np.asarray(edge_index[0], dtype=np.int64)
    dst0 = np.asarray(edge_index[1], dtype=np.int64)

    deg = np.bincount(dst0, minlength=N).astype(np.float32) + 1.0
    dis = (1.0 / np.sqrt(deg)).astype(np.float32)

    # --- node -> (core, superchunk, slot), balancing edge counts per bin
    NSC = int(np.ceil(N / (NCORES * SC)))
    NSC = -(-NSC // NB) * NB          # quarter-aligned superchunks
    NSCQ = NSC // NB
    nbins = NCORES * NSC
    R = NSC * SC                      # padded rows per core
    QR = R // NB                      # rows per core per quarter
    BLOCK = NCORES * QR               # rows per gather bucket
    assert BLOCK <= 32768

    order = np.argsort(-deg, kind="stable")
    k = np.arange(N)
    rnd = k // nbins                      # deal round
    col = k % nbins
    bin_of_sorted = np.where(rnd % 2 == 0, col, nbins - 1 - col)
    bin_id = np.empty(N, dtype=np.int64)
    bin_id[order] = bin_of_sorted
    pos_in_bin = np.empty(N, dtype=np.int64)
    pos_in_bin[order] = rnd

    npb = int(np.ceil(N / nbins))
    assert npb <= SC
    rng = np.random.default_rng(12345)
    perms = np.stack([rng.permutation(SC)[:npb] for _ in range(nbins)])
    slot = perms[bin_id, pos_in_bin]
    core = bin_id // NSC
    sc = bin_id % NSC
    rw = sc * SC + slot                   # core-local LOGICAL row id
    # staged row-within-superchunk: partition-major so a superchunk's 4
    # activation tiles write one contiguous [128, 4, F] block
    stg = (slot % P) * TPS + slot // P
    srow = sc * SC + stg                  # core-local STAGED row id
    node_row = core * R + srow            # padded output row id (staged)
    # quartered gather-source layout: bucket == AllGather chunk
    grow = (sc // NSCQ) * BLOCK + core * QR + (sc % NSCQ) * SC + stg

    # --- messages (edges + self loops), sorted by (cell, slot)
    loop = np.arange(N)
    ms = grow[np.concatenate([src0, loop])]
    md_core = np.concatenate([core[dst0], core[loop]])
    md_rw = np.concatenate([rw[dst0], rw[loop]])
    m_sc = md_rw // SC
    m_slot = md_rw % SC
    m_bkt = ms // BLOCK
    cell = ((md_core * NSC) + m_sc) * NB + m_bkt
    key = cell * SC + m_slot
    o = np.argsort(key, kind="stable")
    ms_s, cell_s, slot_s = ms[o], cell[o], m_slot[o]

    ncells = NCORES * NSC * NB
    cell_starts = np.searchsorted(cell_s, np.arange(ncells))
    cell_ends = np.searchsorted(cell_s, np.arange(ncells) + 1)

    # --- pack cells into groups of <=128 edges spanning < WMAX slots.
    # Window boundaries are SHARED across the 8 cores (close a window when
    # any core reaches 128 edges or the span reaches WMAX), so the PSUM
    # window offsets are compile-time constants — no per-cell register
    # loads on the PE engine.
    groups = [[] for _ in range(ncells)]   # (start, end, lo); may be empty
    lo_list = [[] for _ in range(NSC * NB)]
    for sci in range(NSC):
        for b in range(NB):
            scb = sci * NB + b
            arrs, base = [], []
            for co in range(NCORES):
                c = (co * NSC + sci) * NB + b
                s, e = int(cell_starts[c]), int(cell_ends[c])
                arrs.append(slot_s[s:e])
                base.append(s)
            ptr = [0] * NCORES
            while any(ptr[co] < len(arrs[co]) for co in range(NCORES)):
                lo = min(
                    int(arrs[co][ptr[co]])
                    for co in range(NCORES)
                    if ptr[co] < len(arrs[co])
                )
                lo = min(lo, SC - WMAX)
                hi = lo + WMAX
                for co in range(NCORES):
                    a, p0 = arrs[co], ptr[co]
                    pe_ = int(np.searchsorted(a, hi))
                    if pe_ - p0 > P:
                        hi = int(a[p0 + P])
                assert hi > lo, "slot tie overflow (>128 edges on one slot)"
                for co in range(NCORES):
                    a, p0 = arrs[co], ptr[co]
                    cnt = int(np.searchsorted(a, hi)) - p0
                    assert cnt <= P
                    c = (co * NSC + sci) * NB + b
                    groups[c].append(
                        (base[co] + p0, base[co] + p0 + cnt, lo)
                    )
                    ptr[co] += cnt
                lo_list[scb].append(lo)
    NG = max(1, max(len(g) for g in groups))

    # --- per-core tables
    # idx padding: inside/between real groups pad with 0 (gathered but
    # ignored via zero one-hot rows).  Per gather chunk, the static
    # num_idxs is the max real count across the 8 cores; beyond it idxs
    # are -1 so the DMA skips the common tail.
    chunk_sizes = _chunks(NG)
    nch = len(chunk_sizes)
    cbase = np.concatenate([[0], np.cumsum(chunk_sizes)])  # group offsets
    ncols = NSC * NB * NG
    idx_tab = np.zeros((NCORES, NSC * NB, NG * P), dtype=np.int16)
    srel_tab = np.full((NCORES, ncols, P), -1.0, dtype=np.float16)
    srel0_tab = np.full((NCORES, NSC, P), -1.0, dtype=np.float32)
    cnt_tab = np.ones((NCORES, NSC * NB, nch), dtype=np.int32)
    for c in range(ncells):
        co, rem = divmod(c, NSC * NB)
        scb = rem                    # (sc*NB + b) index
        sci, b = divmod(rem, NB)
        glist = groups[c]
        for g, (s, e, lo) in enumerate(glist):
            n = e - s
            base = scb * NG + g
            idx_tab[co, scb, g * P : g * P + n] = (ms_s[s:e] - b * BLOCK).astype(
                np.int16
            )
            if b == 0 and g == 0:
                srel0_tab[co, sci, :n] = slot_s[s:e].astype(np.float32)
            else:
                srel_tab[co, base, :n] = (slot_s[s:e] - lo).astype(np.float16)
        for ci in range(nch):
            g0, g1 = cbase[ci], cbase[ci + 1]
            cnt_tab[co, scb, ci] = max(min(len(glist), g1) - g0, 0)
    # static per-(cell, chunk) group count = max across cores (>=1 for
    # chunk 0 so the S0 start-matmul always has a gathered tile)
    gcnt = cnt_tab.max(axis=0)                 # [NSC*NB, nch] in groups
    gcnt[:, 0] = np.maximum(gcnt[:, 0], 1)

    # group offsets in consumption order (sci, b, ci) — shared by the
    # pre-gathered layer-1 feature/one-hot streams and their loads
    offs = np.zeros((NSC * NB, nch), dtype=np.int64)
    tot = 0
    for scb in range(NSC * NB):
        for ci in range(nch):
            offs[scb, ci] = tot
            tot += int(gcnt[scb, ci])
    NGTOT = tot
    # per-superchunk stream extents (layer 1 loads one block per sci)
    sci_off = np.array(
        [offs[sci * NB, 0] for sci in range(NSC)] + [NGTOT], dtype=np.int64
    )
    GSMAX = int((sci_off[1:] - sci_off[:-1]).max())

    # layer-1 edge features pre-gathered on the host, partition-major:
    # xg[p, gidx, :] = dis-scaled x of the src of edge (gidx, p), 0 if pad
    xg_rows = np.full((NCORES, NGTOT * P), -1, dtype=np.int64)
    # host-prebuilt one-hot stream, same packing: s1[p, gidx, w]
    s1_tab = np.zeros((NCORES, NGTOT, P, WMAX), dtype=F8)
    one8 = F8(1.0)
    for c in range(ncells):
        co, rem = divmod(c, NSC * NB)
        scb = rem
        _, b = divmod(rem, NB)
        for g, (s, e, lo) in enumerate(groups[c]):
            ci = int(np.searchsorted(cbase, g, "right") - 1)
            pos = offs[scb, ci] + (g - cbase[ci])
            xg_rows[co, pos * P : pos * P + (e - s)] = ms_s[s:e]
            if not (b == 0 and g == 0):
                lanes = np.arange(e - s)
                s1_tab[co, pos, lanes, (slot_s[s:e] - lo)] = one8
    s1 = [
        np.ascontiguousarray(np.transpose(s1_tab[co], (1, 0, 2)))
        for co in range(NCORES)
    ]

    # wrap idx to [16, cols] then replicate to 128 partitions
    idx_wrapped = idx_tab.reshape(NCORES, NSC * NB, NG * P // 16, 16)
    idx_wrapped = np.transpose(idx_wrapped, (0, 1, 3, 2))  # [.., 16, NG*8]
    idx_wrapped = np.tile(idx_wrapped, (1, 1, 8, 1))       # [.., 128, NG*8]
    # final SBUF-layout table per core: [128, NSC*NB*NG*8]
    idx_sb = np.ascontiguousarray(
        np.transpose(idx_wrapped, (0, 2, 1, 3)).reshape(NCORES, P, -1)
    )
    srel_sb = np.ascontiguousarray(np.transpose(srel_tab, (0, 2, 1)))
    srel0_sb = np.ascontiguousarray(np.transpose(srel0_tab, (0, 2, 1)))

    # per-core dis column tables [128, NT] (logical tile-major layout)
    NT = R // P
    dis_pad = np.zeros(NCORES * R, dtype=np.float32)
    dis_pad[core * R + rw] = dis
    dis_sb = np.ascontiguousarray(
        dis_pad.reshape(NCORES, NT, P).transpose(0, 2, 1)
    )
    dis2_sb = np.ascontiguousarray(dis_sb * dis_sb)

    # gather-source xs in the grow layout, pre-scaled by dis
    fdt = F8 if XF8 else np.float16
    xs_g = np.zeros((NCORES * R + 1, F_IN), dtype=fdt)
    xs_g[grow] = (x.astype(np.float32) * dis[:, None]).astype(fdt)
    # pad rows (-1) read the trailing zero row
    xg = [
        np.ascontiguousarray(
            xs_g[xg_rows[co]].reshape(NGTOT, P, F_IN).transpose(1, 0, 2)
        )
        for co in range(NCORES)
    ]

    iota_t = np.tile(np.arange(WMAX, dtype=np.float16), NG)
    iota_t = np.broadcast_to(iota_t, (P, NG * WMAX)).reshape(P, NG, WMAX).copy()
    iota_sc = np.broadcast_to(
        np.arange(SC, dtype=np.float16), (P, SC)
    ).copy()

    return dict(
        N=N, NSC=NSC, NSCQ=NSCQ, R=R, QR=QR, BLOCK=BLOCK, NG=NG, NT=NT,
        node_row=node_row, xg=xg, s1=s1, NGTOT=NGTOT, offs=offs,
        sci_off=sci_off, GSMAX=GSMAX,
        idx_sb=idx_sb, srel_sb=srel_sb, srel0_sb=srel0_sb,
        lo_list=lo_list, gcnt=gcnt,
        dis_sb=dis_sb, dis2_sb=dis2_sb, iota_t=iota_t, iota_sc=iota_sc,
    )


# ------------------------------------------------------------- bass program
def _build(pp, use_prep, b_nonzero):
    import concourse.bass as bass
    import concourse.bacc as bacc
    import concourse.mybir as mybir
    from concourse import tile

    f32 = mybir.dt.float32
    f16 = mybir.dt.float16
    f8 = mybir.dt.float8e4
    i16 = mybir.dt.int16
    fdat1 = f8 if XF8 else f16
    NSC, NSCQ, R, QR, BLOCK = pp["NSC"], pp["NSCQ"], pp["R"], pp["QR"], pp["BLOCK"]
    NG, NT = pp["NG"], pp["NT"]
    ncols = NSC * NB * NG
    chunk_sizes = _chunks(NG)
    gmax = max(chunk_sizes)
    v_g1bufs = int(os.environ.get("GCN2_G1BUFS", "4"))
    v_g2bufs = int(os.environ.get("GCN2_G2BUFS", "16"))

    scratch = int(os.environ.get("GCN2_SCRATCH", str(64 * 1024)))
    v_qn = int(os.environ.get("GCN2_QN", "4"))
    nc = bacc.Bacc(
        "TRN2", target_bir_lowering=False, debug=False, num_devices=NCORES,
        dynamic_dma_scratch_size=scratch, num_swdge_queues=v_qn,
    )

    NGTOT = pp["NGTOT"]
    offs = pp["offs"]
    sci_off = pp["sci_off"]
    GSMAX = pp["GSMAX"]
    xg_d = nc.dram_tensor("xg", [P, NGTOT, F_IN], fdat1, kind="ExternalInput")
    s1_d = nc.dram_tensor("s1t", [P, NGTOT, WMAX], f8, kind="ExternalInput")
    idx_d = nc.dram_tensor("idxt", [P, ncols * 8], i16, kind="ExternalInput")
    srel_d = nc.dram_tensor("srelt", [P, ncols], f16, kind="ExternalInput")
    srel0_d = nc.dram_tensor("srel0t", [P, NSC], f32, kind="ExternalInput")
    lo_list = pp["lo_list"]
    nch = len(chunk_sizes)
    gcnt = pp["gcnt"]
    cbase = [0]
    for gn in chunk_sizes:
        cbase.append(cbase[-1] + gn)
    dis_d = nc.dram_tensor("dist", [P, NT], f32, kind="ExternalInput")
    dis2_d = nc.dram_tensor("dis2t", [P, NT], f32, kind="ExternalInput")
    it_d = nc.dram_tensor("iotat", [P, NG, WMAX], f16, kind="ExternalInput")
    isc_d = nc.dram_tensor("iotasc", [P, SC], f16, kind="ExternalInput")
    W1_d = nc.dram_tensor("W1h", [F_IN, F_HID], f16, kind="ExternalInput")
    b1_d = nc.dram_tensor("b1r", [P, F_HID], f32, kind="ExternalInput")
    W2_d = nc.dram_tensor("W2h", [F_HID, F_OUT], f16, kind="ExternalInput")
    b2_d = nc.dram_tensor("b2r", [P, F_OUT], f32, kind="ExternalInput")
    out_d = nc.dram_tensor("out", [NSC, P, TPS, F_OUT], f32, kind="ExternalOutput")

    u2loc = [
        nc.dram_tensor(f"u2loc{q}", [NSCQ, P, TPS, F_HID], f16) for q in range(NB)
    ]
    u2g = [
        nc.dram_tensor(f"u2g{q}", [BLOCK, F_HID], f16, addr_space="Shared")
        for q in range(NB)
    ]

    dma_sems = [nc.alloc_semaphore(f"gsem{q}") for q in range(v_qn)]

    with tile.TileContext(nc) as tc:
        with (
            tc.tile_pool(name="const", bufs=1) as cpool,
            tc.tile_pool(name="vt", bufs=3) as vpool,
            tc.tile_pool(name="g1", bufs=v_g1bufs) as g1pool,
            tc.tile_pool(name="s1", bufs=v_g1bufs) as s1pool,
            tc.tile_pool(name="g2", bufs=v_g2bufs) as g2pool,
            tc.tile_pool(name="smat", bufs=4) as s2pool,
            tc.tile_pool(name="s0mat", bufs=2) as s0pool,
            tc.tile_pool(name="uwork", bufs=3) as upool,
            tc.tile_pool(name="psagg", bufs=4, space="PSUM") as pagg,
            tc.tile_pool(name="psmm", bufs=2, space="PSUM") as pmm,
        ):
            # ---- constants / tables resident in SBUF
            idx_sb = cpool.tile([P, ncols * 8], i16)
            srel_sb = cpool.tile([P, ncols], f16)
            srel0_sb = cpool.tile([P, NSC], f32)
            dis_sb = cpool.tile([P, NT], f32)
            dis2_sb = cpool.tile([P, NT], f32)
            it_sb = cpool.tile([P, NG, WMAX], f16)
            isc_sb = cpool.tile([P, SC], f16)
            W1_sb = cpool.tile([F_IN, F_HID], f16)
            b1_sb = cpool.tile([P, F_HID], f32)
            W2_sb = cpool.tile([F_HID, F_OUT], f16)
            b2_sb = cpool.tile([P, F_OUT], f32)
            # small layer-1 tables first so the xg stream starts ASAP;
            # the big layer-2-only idx table loads on the scalar queue in
            # parallel (only the L2 gather gen waits on it)
            for sb, d in [
                (srel0_sb, srel0_d), (dis_sb, dis_d), (dis2_sb, dis2_d),
                (isc_sb, isc_d), (W1_sb, W1_d), (b1_sb, b1_d),
                (W2_sb, W2_d), (b2_sb, b2_d),
            ]:
                nc.sync.dma_start(sb[:], d[:])
            for sb, d in [(idx_sb, idx_d), (srel_sb, srel_d), (it_sb, it_d)]:
                nc.scalar.dma_start(sb[:], d[:])

            qctr = [0]

            def load_l2(scb, ci, ge):
                b = scb % NB
                g0 = cbase[ci]
                q = qctr[0] % v_qn
                qctr[0] += 1
                gt = g2pool.tile([P, gmax, F_IN], f16, tag="g2")
                args = dict(elem_step=F_IN, queue_num=q)
                if use_prep:
                    args.update(prepare_only=True, sem=dma_sems[q])
                nc.gpsimd.dma_gather(
                    gt[:, :ge, :],
                    u2g[b][:],
                    idx_sb[:, (scb * NG + g0) * 8 : (scb * NG + g0 + ge) * 8],
                    ge * P, ge * P, F_IN, **args,
                )
                if use_prep:
                    nc.gpsimd.trigger_dma(count=None, queue_num=q)
                return gt

            def agg_layer(layer, out_cb):
                """out_cb(sci, ps) with ps = (A_hat @ src)^T for superchunk.

                out_cb(sci) is emitted after agg(sci+1) so the PE never
                stalls on the PSUM->SBUF copy at superchunk boundaries."""
                s0dt = fdat1 if layer == 1 else f16
                pending = None
                for sci in range(NSC):
                    ps = pagg.tile([P, SC], f32)
                    ngrp = sum(
                        int(gcnt[sci * NB + b, ci])
                        for b in range(NB)
                        for ci in range(nch)
                    )
                    gi = 0
                    if layer == 1:
                        # one bulk load pair for the whole superchunk
                        off0 = int(sci_off[sci])
                        gs = int(sci_off[sci + 1]) - off0
                        gt1 = g1pool.tile([P, GSMAX, F_IN], fdat1, tag="g1")
                        st1 = s1pool.tile([P, GSMAX, WMAX], f8, tag="s1")
                        nc.sync.dma_start(
                            gt1[:, :gs, :], xg_d[:, off0 : off0 + gs, :]
                        )
                        nc.scalar.dma_start(
                            st1[:, :gs, :], s1_d[:, off0 : off0 + gs, :]
                        )
                    for b in range(NB):
                        scb = sci * NB + b
                        tiles = {}
                        if layer == 2:
                            for ci in range(nch):
                                ge = int(gcnt[scb, ci])
                                if ge == 0:
                                    continue
                                tiles[ci] = load_l2(scb, ci, ge)
                            # one-hot matrices for the whole cell in one op
                            S2 = s2pool.tile([P, NG, WMAX], f16, tag="s")
                            nc.vector.tensor_tensor(
                                S2[:],
                                it_sb[:],
                                srel_sb[:, scb * NG : (scb + 1) * NG]
                                .unsqueeze(2)
                                .broadcast_to((P, NG, WMAX)),
                                op=mybir.AluOpType.is_equal,
                            )
                        if b == 0:
                            S0 = s0pool.tile([P, SC], s0dt, tag=f"s0l{layer}")
                            nc.vector.tensor_scalar(
                                S0[:],
                                isc_sb[:],
                                srel0_sb[:, sci : sci + 1],
                                None,
                                op0=mybir.AluOpType.is_equal,
                            )
                        for ci in range(nch):
                            ge = int(gcnt[scb, ci])
                            for gg in range(ge):
                                g = cbase[ci] + gg
                                if layer == 1:
                                    pos = int(offs[scb, ci]) + gg - off0
                                    lhs = gt1[:, pos, :]
                                    rhs = st1[:, pos, :]
                                else:
                                    lhs = tiles[ci][:, gg, :]
                                    rhs = S2[:, g, :]
                                if b == 0 and g == 0:
                                    nc.tensor.matmul(
                                        ps[:, :],
                                        lhs,
                                        S0[:],
                                        start=True,
                                        stop=(gi == ngrp - 1),
                                    )
                                else:
                                    lo = lo_list[scb][g]
                                    nc.tensor.matmul(
                                        ps[:, lo : lo + WMAX],
                                        lhs,
                                        rhs,
                                        start=False,
                                        stop=(gi == ngrp - 1),
                                    )
                                gi += 1
                    if pending is not None:
                        out_cb(*pending)
                    pending = (sci, ps)
                out_cb(*pending)

            # ---------------- layer 1
            def l1_out(sci, ps):
                v = vpool.tile([P, SC], f16, tag="v")
                nc.scalar.copy(v[:], ps[:])
                ust = upool.tile([P, TPS, F_HID], f16, tag="u")
                for j in range(TPS):
                    t = sci * TPS + j
                    pb = pmm.tile([P, F_HID], f32, tag="pb")
                    nc.tensor.matmul(
                        pb[:], v[:, j * P : (j + 1) * P], W1_sb[:],
                        start=True, stop=True,
                    )
                    if b_nonzero:
                        w = upool.tile([P, F_HID], f32, tag="w")
                        nc.vector.tensor_scalar(
                            w[:], pb[:], dis_sb[:, t : t + 1], None,
                            op0=mybir.AluOpType.mult,
                        )
                        nc.vector.tensor_tensor(
                            w[:], w[:], b1_sb[:], op=mybir.AluOpType.add
                        )
                        nc.scalar.activation(
                            ust[:, j, :], w[:],
                            mybir.ActivationFunctionType.Relu,
                            scale=dis_sb[:, t : t + 1],
                        )
                    else:
                        # u = dis * relu(dis*agg@W1) = relu(dis^2 * agg@W1)
                        nc.scalar.activation(
                            ust[:, j, :], pb[:],
                            mybir.ActivationFunctionType.Relu,
                            scale=dis2_sb[:, t : t + 1],
                        )
                q, scq = divmod(sci, NSCQ)
                nc.sync.dma_start(u2loc[q][scq], ust[:])
                if scq == NSCQ - 1:
                    nc.gpsimd.collective_compute(
                        "AllGather",
                        mybir.AluOpType.bypass,
                        replica_groups=[list(range(NCORES))],
                        ins=[u2loc[q][:]],
                        outs=[u2g[q][:]],
                    )

            agg_layer(1, l1_out)

            # ---------------- layer 2
            def l2_out(sci, ps):
                v = vpool.tile([P, SC], f16, tag="v")
                nc.scalar.copy(v[:], ps[:])
                yst = upool.tile([P, TPS, F_OUT], f32, tag="y")
                for j in range(TPS):
                    t = sci * TPS + j
                    pb = pmm.tile([P, F_OUT], f32, tag="pe")
                    nc.tensor.matmul(
                        pb[:], v[:, j * P : (j + 1) * P], W2_sb[:],
                        start=True, stop=True,
                    )
                    if b_nonzero:
                        y = upool.tile([P, F_OUT], f32, tag="yb")
                        nc.vector.tensor_scalar(
                            y[:], pb[:], dis_sb[:, t : t + 1], None,
                            op0=mybir.AluOpType.mult,
                        )
                        nc.vector.tensor_tensor(
                            yst[:, j, :], y[:], b2_sb[:],
                            op=mybir.AluOpType.add,
                        )
                    else:
                        nc.scalar.activation(
                            yst[:, j, :], pb[:],
                            mybir.ActivationFunctionType.Copy,
                            scale=dis_sb[:, t : t + 1],
                        )
                nc.sync.dma_start(out_d[sci], yst[:])

            agg_layer(2, l2_out)

    nc.compile()
    return nc


# ------------------------------------------------------------------ driver
_CACHE = {}
TRACE = False
LAST_RESULTS = None


def kernel(x, edge_index, W1, b1, W2, b2):
    from concourse.bass_utils import run_bass_kernel_spmd

    x = np.asarray(x)
    edge_index = np.asarray(edge_index)
    W1 = np.asarray(W1, dtype=np.float32)
    b1 = np.asarray(b1, dtype=np.float32)
    W2 = np.asarray(W2, dtype=np.float32)
    b2 = np.asarray(b2, dtype=np.float32)

    use_prep = os.environ.get("GCN2_PREP", "0") == "1"
    b_nonzero = bool(np.any(b1) or np.any(b2))
    pp = _prep(x, edge_index)
    key = (
        x.shape, edge_index.shape, pp["NG"], use_prep, b_nonzero,
        os.environ.get("GCN2_QN", "4"),
    )
    if key not in _CACHE:
        _CACHE[key] = _build(pp, use_prep, b_nonzero)
    nc = _CACHE[key]

    b1r = np.broadcast_to(b1, (P, F_HID)).copy()
    b2r = np.broadcast_to(b2, (P, F_OUT)).copy()
    in_maps = []
    for c in range(NCORES):
        m = {
            "xg": pp["xg"][c],
            "s1t": pp["s1"][c],
            "idxt": pp["idx_sb"][c],
            "srelt": pp["srel_sb"][c],
            "srel0t": pp["srel0_sb"][c],
            "dist": pp["dis_sb"][c],
            "dis2t": pp["dis2_sb"][c],
            "iotat": pp["iota_t"],
            "iotasc": pp["iota_sc"],
            "W1h": W1.astype(np.float16),
            "b1r": b1r,
            "W2h": W2.astype(np.float16),
            "b2r": b2r,
        }
        in_maps.append(m)
    res = run_bass_kernel_spmd(
        nc, in_maps, list(range(NCORES)), trace=TRACE
    )
    global LAST_RESULTS
    LAST_RESULTS = res
    outs = np.stack(
        [np.asarray(r["out"]).reshape(pp["R"], F_OUT) for r in res.results]
    )  # [C, R, FO]
    outs = outs.reshape(NCORES * pp["R"], F_OUT)
    return np.ascontiguousarray(outs[pp["node_row"]])


# revision 20
# speedup vs baseline: 1.0408x; 1.0408x over previous
"""2-layer GCN on 8 Trainium2 NeuronCores (Bass/Tile SPMD kernel).

Math: reference computes, per layer,
    out = A_norm @ (in @ W) + b,   A_norm[d,s] = dis[d]*dis[s]*A_hat[d,s]
with A_hat = adjacency + self-loops, dis = 1/sqrt(deg).
We use associativity to aggregate first:
    out = dis ⊙ (A_hat @ (dis ⊙ in)) @ W + b
so the per-edge work is a pure gather+segment-sum of pre-scaled node
features (no per-edge multiplies).

Sharding: nodes are balanced across 8 cores x NSC superchunks of 512
"slots" each (NSC padded to a multiple of NB so AllGather quarters are
superchunk-aligned).  Edges are assigned to the core/superchunk of their
dst node, bucketed by src block (so gather indices fit in int16), sorted
by dst slot, and packed into 128-edge groups.  Each group is one one-hot
matmul G.T @ S that segment-sums the group into PSUM[feat, 512 slots].

Layer 1 consumes a HOST-pre-gathered fp8 edge-feature stream (xg) plus a
HOST-prebuilt fp8 one-hot stream (S) — both pure sequential DMA, no
on-chip gather or S-build.  Layer 2 gathers the fp16 hidden features
with pipelined dma_gather (prepare_only + per-queue trigger/sems) from
the AllGathered hidden state; its one-hot S is built on the DVE
(is_equal against an iota table) since the gathered operand is fp16.

Per-node output rows are stored "staged" (row = p*4 + j for slot
j*128+p) so each superchunk's four activation tiles land in one
contiguous [128, 4, F] DMA.

The gather-source row space is quartered: row = q*BLOCK + core*QR + o
so that each of 4 chunked AllGathers of the hidden layer delivers
exactly one gather bucket, letting layer-2 aggregation start while
later chunks are still in flight.
"""

import os
import sys

import ml_dtypes
import numpy as np

sys.path.insert(0, "/opt/trn_rl_repo")

P = 128          # partitions / group size
SC = 512         # slots per superchunk (= one PSUM bank of f32)
NCORES = 8
F_IN = 128
F_HID = 128
F_OUT = 64
NB = 4           # src buckets == AllGather chunks (int16 gather idx limit)
TPS = SC // P    # tiles per superchunk (4)

WMAX = int(os.environ.get("GCN2_WMAX", "64"))   # max slot span of a group
GCHUNK = int(os.environ.get("GCN2_L2G", "8"))  # ucode cap: 1024 idxs/call  # groups per gather call
XF8 = os.environ.get("GCN2_XF8", "1") == "1"    # fp8 layer-1 streams

F8 = ml_dtypes.float8_e4m3


def _chunks(ng):
    n = -(-ng // GCHUNK)
    base = ng // n
    rem = ng % n
    return [base + (1 if i < rem else 0) for i in range(n)]


# ----------------------------------------------------------------- host prep
def _prep(x, edge_index):
    N, F = x.shape
    assert F == F_IN
    src0 = np.asarray(edge_index[0], dtype=np.int64)
    dst0 = np.asarray(edge_index[1], dtype=np.int64)

    deg = np.bincount(dst0, minlength=N).astype(np.float32) + 1.0
    dis = (1.0 / np.sqrt(deg)).astype(np.float32)

    # --- node -> (core, superchunk, slot), balancing edge counts per bin
    NSC = int(np.ceil(N / (NCORES * SC)))
    NSC = -(-NSC // NB) * NB          # quarter-aligned superchunks
    NSCQ = NSC // NB
    nbins = NCORES * NSC
    R = NSC * SC                      # padded rows per core
    QR = R // NB                      # rows per core per quarter
    BLOCK = NCORES * QR               # rows per gather bucket
    assert BLOCK <= 32768

    order = np.argsort(-deg, kind="stable")
    k = np.arange(N)
    rnd = k // nbins                      # deal round
    col = k % nbins
    bin_of_sorted = np.where(rnd % 2 == 0, col, nbins - 1 - col)
    bin_id = np.empty(N, dtype=np.int64)
    bin_id[order] = bin_of_sorted
    pos_in_bin = np.empty(N, dtype=np.int64)
    pos_in_bin[order] = rnd

    npb = int(np.ceil(N / nbins))
    assert npb <= SC
    rng = np.random.default_rng(12345)
    perms = np.stack([rng.permutation(SC)[:npb] for _ in range(nbins)])
    slot = perms[bin_id, pos_in_bin]
    core = bin_id // NSC
    sc = bin_id % NSC
    rw = sc * SC + slot                   # core-local LOGICAL row id
    # staged row-within-superchunk: partition-major so a superchunk's 4
    # activation tiles write one contiguous [128, 4, F] block
    stg = (slot % P) * TPS + slot // P
    srow = sc * SC + stg                  # core-local STAGED row id
    node_row = core * R + srow            # padded output row id (staged)
    # quartered gather-source layout: bucket == AllGather chunk
    grow = (sc // NSCQ) * BLOCK + core * QR + (sc % NSCQ) * SC + stg

    # --- messages (edges + self loops), sorted by (cell, slot)
    loop = np.arange(N)
    ms = grow[np.concatenate([src0, loop])]
    md_core = np.concatenate([core[dst0], core[loop]])
    md_rw = np.concatenate([rw[dst0], rw[loop]])
    m_sc = md_rw // SC
    m_slot = md_rw % SC
    m_bkt = ms // BLOCK
    cell = ((md_core * NSC) + m_sc) * NB + m_bkt
    key = cell * SC + m_slot
    o = np.argsort(key, kind="stable")
    ms_s, cell_s, slot_s = ms[o], cell[o], m_slot[o]

    ncells = NCORES * NSC * NB
    cell_starts = np.searchsorted(cell_s, np.arange(ncells))
    cell_ends = np.searchsorted(cell_s, np.arange(ncells) + 1)

    # --- pack cells into groups of <=128 edges spanning < WMAX slots.
    # Window boundaries are SHARED across the 8 cores (close a window when
    # any core reaches 128 edges or the span reaches WMAX), so the PSUM
    # window offsets are compile-time constants — no per-cell register
    # loads on the PE engine.
    groups = [[] for _ in range(ncells)]   # (start, end, lo); may be empty
    lo_list = [[] for _ in range(NSC * NB)]
    for sci in range(NSC):
        for b in range(NB):
            scb = sci * NB + b
            arrs, base = [], []
            for co in range(NCORES):
                c = (co * NSC + sci) * NB + b
                s, e = int(cell_starts[c]), int(cell_ends[c])
                arrs.append(slot_s[s:e])
                base.append(s)
            ptr = [0] * NCORES
            while any(ptr[co] < len(arrs[co]) for co in range(NCORES)):
                lo = min(
                    int(arrs[co][ptr[co]])
                    for co in range(NCORES)
                    if ptr[co] < len(arrs[co])
                )
                lo = min(lo, SC - WMAX)
                hi = lo + WMAX
                for co in range(NCORES):
                    a, p0 = arrs[co], ptr[co]
                    pe_ = int(np.searchsorted(a, hi))
                    if pe_ - p0 > P:
                        hi = int(a[p0 + P])
                assert hi > lo, "slot tie overflow (>128 edges on one slot)"
                for co in range(NCORES):
                    a, p0 = arrs[co], ptr[co]
                    cnt = int(np.searchsorted(a, hi)) - p0
                    assert cnt <= P
                    c = (co * NSC + sci) * NB + b
                    groups[c].append(
                        (base[co] + p0, base[co] + p0 + cnt, lo)
                    )
                    ptr[co] += cnt
                lo_list[scb].append(lo)
    NG = max(1, max(len(g) for g in groups))

    # --- per-core tables
    # idx padding: inside/between real groups pad with 0 (gathered but
    # ignored via zero one-hot rows).  Per gather chunk, the static
    # num_idxs is the max real count across the 8 cores; beyond it idxs
    # are -1 so the DMA skips the common tail.
    chunk_sizes = _chunks(NG)
    nch = len(chunk_sizes)
    cbase = np.concatenate([[0], np.cumsum(chunk_sizes)])  # group offsets
    ncols = NSC * NB * NG
    idx_tab = np.zeros((NCORES, NSC * NB, NG * P), dtype=np.int16)
    srel_tab = np.full((NCORES, ncols, P), -1.0, dtype=np.float16)
    srel0_tab = np.full((NCORES, NSC, P), -1.0, dtype=np.float32)
    cnt_tab = np.ones((NCORES, NSC * NB, nch), dtype=np.int32)
    for c in range(ncells):
        co, rem = divmod(c, NSC * NB)
        scb = rem                    # (sc*NB + b) index
        sci, b = divmod(rem, NB)
        glist = groups[c]
        for g, (s, e, lo) in enumerate(glist):
            n = e - s
            base = scb * NG + g
            idx_tab[co, scb, g * P : g * P + n] = (ms_s[s:e] - b * BLOCK).astype(
                np.int16
            )
            if b == 0 and g == 0:
                srel0_tab[co, sci, :n] = slot_s[s:e].astype(np.float32)
            else:
                srel_tab[co, base, :n] = (slot_s[s:e] - lo).astype(np.float16)
        for ci in range(nch):
            g0, g1 = cbase[ci], cbase[ci + 1]
            cnt_tab[co, scb, ci] = max(min(len(glist), g1) - g0, 0)
    # static per-(cell, chunk) group count = max across cores (>=1 for
    # chunk 0 so the S0 start-matmul always has a gathered tile)
    gcnt = cnt_tab.max(axis=0)                 # [NSC*NB, nch] in groups
    gcnt[:, 0] = np.maximum(gcnt[:, 0], 1)

    # group offsets in consumption order (sci, b, ci) — shared by the
    # pre-gathered layer-1 feature/one-hot streams and their loads
    offs = np.zeros((NSC * NB, nch), dtype=np.int64)
    tot = 0
    for scb in range(NSC * NB):
        for ci in range(nch):
            offs[scb, ci] = tot
            tot += int(gcnt[scb, ci])
    NGTOT = tot
    # per-superchunk stream extents (layer 1 loads one block per sci)
    sci_off = np.array(
        [offs[sci * NB, 0] for sci in range(NSC)] + [NGTOT], dtype=np.int64
    )
    GSMAX = int((sci_off[1:] - sci_off[:-1]).max())

    # layer-1 edge features pre-gathered on the host, partition-major:
    # xg[p, gidx, :] = dis-scaled x of the src of edge (gidx, p), 0 if pad
    xg_rows = np.full((NCORES, NGTOT * P), -1, dtype=np.int64)
    # host-prebuilt one-hot stream, same packing: s1[p, gidx, w]
    s1_tab = np.zeros((NCORES, NGTOT, P, WMAX), dtype=F8)
    one8 = F8(1.0)
    for c in range(ncells):
        co, rem = divmod(c, NSC * NB)
        scb = rem
        _, b = divmod(rem, NB)
        for g, (s, e, lo) in enumerate(groups[c]):
            ci = int(np.searchsorted(cbase, g, "right") - 1)
            pos = offs[scb, ci] + (g - cbase[ci])
            xg_rows[co, pos * P : pos * P + (e - s)] = ms_s[s:e]
            if not (b == 0 and g == 0):
                lanes = np.arange(e - s)
                s1_tab[co, pos, lanes, (slot_s[s:e] - lo)] = one8
    s1 = [
        np.ascontiguousarray(np.transpose(s1_tab[co], (1, 0, 2)))
        for co in range(NCORES)
    ]

    # wrap idx to [16, cols] then replicate to 128 partitions
    idx_wrapped = idx_tab.reshape(NCORES, NSC * NB, NG * P // 16, 16)
    idx_wrapped = np.transpose(idx_wrapped, (0, 1, 3, 2))  # [.., 16, NG*8]
    idx_wrapped = np.tile(idx_wrapped, (1, 1, 8, 1))       # [.., 128, NG*8]
    # final SBUF-layout table per core: [128, NSC*NB*NG*8]
    idx_sb = np.ascontiguousarray(
        np.transpose(idx_wrapped, (0, 2, 1, 3)).reshape(NCORES, P, -1)
    )
    srel_sb = np.ascontiguousarray(np.transpose(srel_tab, (0, 2, 1)))
    srel0_sb = np.ascontiguousarray(np.transpose(srel0_tab, (0, 2, 1)))

    # per-core dis column tables [128, NT] (logical tile-major layout)
    NT = R // P
    dis_pad = np.zeros(NCORES * R, dtype=np.float32)
    dis_pad[core * R + rw] = dis
    dis_sb = np.ascontiguousarray(
        dis_pad.reshape(NCORES, NT, P).transpose(0, 2, 1)
    )
    dis2_sb = np.ascontiguousarray(dis_sb * dis_sb)

    # gather-source xs in the grow layout, pre-scaled by dis
    fdt = F8 if XF8 else np.float16
    xs_g = np.zeros((NCORES * R + 1, F_IN), dtype=fdt)
    xs_g[grow] = (x.astype(np.float32) * dis[:, None]).astype(fdt)
    # pad rows (-1) read the trailing zero row
    xg = [
        np.ascontiguousarray(
            xs_g[xg_rows[co]].reshape(NGTOT, P, F_IN).transpose(1, 0, 2)
        )
        for co in range(NCORES)
    ]

    iota_t = np.tile(np.arange(WMAX, dtype=np.float16), NG)
    iota_t = np.broadcast_to(iota_t, (P, NG * WMAX)).reshape(P, NG, WMAX).copy()
    iota_sc = np.broadcast_to(
        np.arange(SC, dtype=np.float16), (P, SC)
    ).copy()

    return dict(
        N=N, NSC=NSC, NSCQ=NSCQ, R=R, QR=QR, BLOCK=BLOCK, NG=NG, NT=NT,
        node_row=node_row, xg=xg, s1=s1, NGTOT=NGTOT, offs=offs,
        sci_off=sci_off, GSMAX=GSMAX,
        idx_sb=idx_sb, srel_sb=srel_sb, srel0_sb=srel0_sb,
        lo_list=lo_list, gcnt=gcnt,
        dis_sb=dis_sb, dis2_sb=dis2_sb, iota_t=iota_t, iota_sc=iota_sc,
    )


# ------------------------------------------------------------- bass program
def _build(pp, use_prep, b_nonzero):
    import concourse.bass as bass
    import concourse.bacc as bacc
    import concourse.mybir as mybir
    from concourse import tile

    f32 = mybir.dt.float32
    f16 = mybir.dt.float16
    f8 = mybir.dt.float8e4
    i16 = mybir.dt.int16
    fdat1 = f8 if XF8 else f16
    NSC, NSCQ, R, QR, BLOCK = pp["NSC"], pp["NSCQ"], pp["R"], pp["QR"], pp["BLOCK"]
    NG, NT = pp["NG"], pp["NT"]
    ncols = NSC * NB * NG
    chunk_sizes = _chunks(NG)
    gmax = max(chunk_sizes)
    v_g1bufs = int(os.environ.get("GCN2_G1BUFS", "3"))
    v_g2bufs = int(os.environ.get("GCN2_G2BUFS", "16"))

    scratch = int(os.environ.get("GCN2_SCRATCH", str(64 * 1024)))
    v_qn = int(os.environ.get("GCN2_QN", "4"))
    nc = bacc.Bacc(
        "TRN2", target_bir_lowering=False, debug=False, num_devices=NCORES,
        dynamic_dma_scratch_size=scratch, num_swdge_queues=v_qn,
    )

    NGTOT = pp["NGTOT"]
    offs = pp["offs"]
    sci_off = pp["sci_off"]
    GSMAX = pp["GSMAX"]
    xg_d = nc.dram_tensor("xg", [P, NGTOT, F_IN], fdat1, kind="ExternalInput")
    s1_d = nc.dram_tensor("s1t", [P, NGTOT, WMAX], f8, kind="ExternalInput")
    idx_d = nc.dram_tensor("idxt", [P, ncols * 8], i16, kind="ExternalInput")
    srel_d = nc.dram_tensor("srelt", [P, ncols], f16, kind="ExternalInput")
    srel0_d = nc.dram_tensor("srel0t", [P, NSC], f32, kind="ExternalInput")
    lo_list = pp["lo_list"]
    nch = len(chunk_sizes)
    gcnt = pp["gcnt"]
    cbase = [0]
    for gn in chunk_sizes:
        cbase.append(cbase[-1] + gn)
    dis_d = nc.dram_tensor("dist", [P, NT], f32, kind="ExternalInput")
    dis2_d = nc.dram_tensor("dis2t", [P, NT], f32, kind="ExternalInput")
    it_d = nc.dram_tensor("iotat", [P, NG, WMAX], f16, kind="ExternalInput")
    isc_d = nc.dram_tensor("iotasc", [P, SC], f16, kind="ExternalInput")
    W1_d = nc.dram_tensor("W1h", [F_IN, F_HID], f16, kind="ExternalInput")
    b1_d = nc.dram_tensor("b1r", [P, F_HID], f32, kind="ExternalInput")
    W2_d = nc.dram_tensor("W2h", [F_HID, F_OUT], f16, kind="ExternalInput")
    b2_d = nc.dram_tensor("b2r", [P, F_OUT], f32, kind="ExternalInput")
    out_d = nc.dram_tensor("out", [NSC, P, TPS, F_OUT], f32, kind="ExternalOutput")

    u2loc = [
        nc.dram_tensor(f"u2loc{q}", [NSCQ, P, TPS, F_HID], f16) for q in range(NB)
    ]
    u2g = [
        nc.dram_tensor(f"u2g{q}", [BLOCK, F_HID], f16, addr_space="Shared")
        for q in range(NB)
    ]

    dma_sems = [nc.alloc_semaphore(f"gsem{q}") for q in range(v_qn)]

    with tile.TileContext(nc) as tc:
        with (
            tc.tile_pool(name="const", bufs=1) as cpool,
            tc.tile_pool(name="vt", bufs=3) as vpool,
            tc.tile_pool(name="g1", bufs=v_g1bufs) as g1pool,
            tc.tile_pool(name="s1", bufs=v_g1bufs) as s1pool,
            tc.tile_pool(name="g2", bufs=v_g2bufs) as g2pool,
            tc.tile_pool(name="smat", bufs=4) as s2pool,
            tc.tile_pool(name="s0mat", bufs=2) as s0pool,
            tc.tile_pool(name="uwork", bufs=3) as upool,
            tc.tile_pool(name="psagg", bufs=4, space="PSUM") as pagg,
            tc.tile_pool(name="psmm", bufs=2, space="PSUM") as pmm,
        ):
            # ---- constants / tables resident in SBUF
            idx_sb = cpool.tile([P, ncols * 8], i16)
            srel_sb = cpool.tile([P, ncols], f16)
            srel0_sb = cpool.tile([P, NSC], f32)
            dis_sb = cpool.tile([P, NT], f32)
            dis2_sb = cpool.tile([P, NT], f32)
            it_sb = cpool.tile([P, NG, WMAX], f16)
            isc_sb = cpool.tile([P, SC], f16)
            W1_sb = cpool.tile([F_IN, F_HID], f16)
            b1_sb = cpool.tile([P, F_HID], f32)
            W2_sb = cpool.tile([F_HID, F_OUT], f16)
            b2_sb = cpool.tile([P, F_OUT], f32)
            for sb, d in [
                (idx_sb, idx_d), (srel_sb, srel_d), (srel0_sb, srel0_d),
                (dis_sb, dis_d), (dis2_sb, dis2_d), (isc_sb, isc_d),
                (W1_sb, W1_d), (b1_sb, b1_d), (W2_sb, W2_d), (b2_sb, b2_d),
                (it_sb, it_d),
            ]:
                nc.sync.dma_start(sb[:], d[:])

            qctr = [0]

            def load_l2(scb, ci, ge):
                b = scb % NB
                g0 = cbase[ci]
                q = qctr[0] % v_qn
                qctr[0] += 1
                gt = g2pool.tile([P, gmax, F_IN], f16, tag="g2")
                args = dict(elem_step=F_IN, queue_num=q)
                if use_prep:
                    args.update(prepare_only=True, sem=dma_sems[q])
                nc.gpsimd.dma_gather(
                    gt[:, :ge, :],
                    u2g[b][:],
                    idx_sb[:, (scb * NG + g0) * 8 : (scb * NG + g0 + ge) * 8],
                    ge * P, ge * P, F_IN, **args,
                )
                if use_prep:
                    nc.gpsimd.trigger_dma(count=None, queue_num=q)
                return gt

            def agg_layer(layer, out_cb):
                """out_cb(sci, ps) with ps = (A_hat @ src)^T for superchunk."""
                s0dt = fdat1 if layer == 1 else f16
                for sci in range(NSC):
                    ps = pagg.tile([P, SC], f32)
                    ngrp = sum(
                        int(gcnt[sci * NB + b, ci])
                        for b in range(NB)
                        for ci in range(nch)
                    )
                    gi = 0
                    if layer == 1:
                        # one bulk load pair for the whole superchunk
                        off0 = int(sci_off[sci])
                        gs = int(sci_off[sci + 1]) - off0
                        gt1 = g1pool.tile([P, GSMAX, F_IN], fdat1, tag="g1")
                        st1 = s1pool.tile([P, GSMAX, WMAX], f8, tag="s1")
                        nc.sync.dma_start(
                            gt1[:, :gs, :], xg_d[:, off0 : off0 + gs, :]
                        )
                        nc.scalar.dma_start(
                            st1[:, :gs, :], s1_d[:, off0 : off0 + gs, :]
                        )
                    for b in range(NB):
                        scb = sci * NB + b
                        tiles = {}
                        if layer == 2:
                            for ci in range(nch):
                                ge = int(gcnt[scb, ci])
                                if ge == 0:
                                    continue
                                tiles[ci] = load_l2(scb, ci, ge)
                            # one-hot matrices for the whole cell in one op
                            S2 = s2pool.tile([P, NG, WMAX], f16, tag="s")
                            nc.vector.tensor_tensor(
                                S2[:],
                                it_sb[:],
                                srel_sb[:, scb * NG : (scb + 1) * NG]
                                .unsqueeze(2)
                                .broadcast_to((P, NG, WMAX)),
                                op=mybir.AluOpType.is_equal,
                            )
                        if b == 0:
                            S0 = s0pool.tile([P, SC], s0dt, tag=f"s0l{layer}")
                            nc.vector.tensor_scalar(
                                S0[:],
                                isc_sb[:],
                                srel0_sb[:, sci : sci + 1],
                                None,
                                op0=mybir.AluOpType.is_equal,
                            )
                        for ci in range(nch):
                            ge = int(gcnt[scb, ci])
                            for gg in range(ge):
                                g = cbase[ci] + gg
                                if layer == 1:
                                    pos = int(offs[scb, ci]) + gg - off0
                                    lhs = gt1[:, pos, :]
                                    rhs = st1[:, pos, :]
                                else:
                                    lhs = tiles[ci][:, gg, :]
                                    rhs = S2[:, g, :]
                                if b == 0 and g == 0:
                                    nc.tensor.matmul(
                                        ps[:, :],
                                        lhs,
                                        S0[:],
                                        start=True,
                                        stop=(gi == ngrp - 1),
                                    )
                                else:
                                    lo = lo_list[scb][g]
                                    nc.tensor.matmul(
                                        ps[:, lo : lo + WMAX],
                                        lhs,
                                        rhs,
                                        start=False,
                                        stop=(gi == ngrp - 1),
                                    )
                                gi += 1
                    out_cb(sci, ps)

            # ---------------- layer 1
            def l1_out(sci, ps):
                v = vpool.tile([P, SC], f16, tag="v")
                nc.scalar.copy(v[:], ps[:])
                ust = upool.tile([P, TPS, F_HID], f16, tag="u")
                for j in range(TPS):
                    t = sci * TPS + j
                    pb = pmm.tile([P, F_HID], f32, tag="pb")
                    nc.tensor.matmul(
                        pb[:], v[:, j * P : (j + 1) * P], W1_sb[:],
                        start=True, stop=True,
                    )
                    if b_nonzero:
                        w = upool.tile([P, F_HID], f32, tag="w")
                        nc.vector.tensor_scalar(
                            w[:], pb[:], dis_sb[:, t : t + 1], None,
                            op0=mybir.AluOpType.mult,
                        )
                        nc.vector.tensor_tensor(
                            w[:], w[:], b1_sb[:], op=mybir.AluOpType.add
                        )
                        nc.scalar.activation(
                            ust[:, j, :], w[:],
                            mybir.ActivationFunctionType.Relu,
                            scale=dis_sb[:, t : t + 1],
                        )
                    else:
                        # u = dis * relu(dis*agg@W1) = relu(dis^2 * agg@W1)
                        nc.scalar.activation(
                            ust[:, j, :], pb[:],
                            mybir.ActivationFunctionType.Relu,
                            scale=dis2_sb[:, t : t + 1],
                        )
                q, scq = divmod(sci, NSCQ)
                nc.sync.dma_start(u2loc[q][scq], ust[:])
                if scq == NSCQ - 1:
                    nc.gpsimd.collective_compute(
                        "AllGather",
                        mybir.AluOpType.bypass,
                        replica_groups=[list(range(NCORES))],
                        ins=[u2loc[q][:]],
                        outs=[u2g[q][:]],
                    )

            agg_layer(1, l1_out)

            # ---------------- layer 2
            def l2_out(sci, ps):
                v = vpool.tile([P, SC], f16, tag="v")
                nc.scalar.copy(v[:], ps[:])
                yst = upool.tile([P, TPS, F_OUT], f32, tag="y")
                for j in range(TPS):
                    t = sci * TPS + j
                    pb = pmm.tile([P, F_OUT], f32, tag="pe")
                    nc.tensor.matmul(
                        pb[:], v[:, j * P : (j + 1) * P], W2_sb[:],
                        start=True, stop=True,
                    )
                    if b_nonzero:
                        y = upool.tile([P, F_OUT], f32, tag="yb")
                        nc.vector.tensor_scalar(
                            y[:], pb[:], dis_sb[:, t : t + 1], None,
                            op0=mybir.AluOpType.mult,
                        )
                        nc.vector.tensor_tensor(
                            yst[:, j, :], y[:], b2_sb[:],
                            op=mybir.AluOpType.add,
                        )
                    else:
                        nc.scalar.activation(
                            yst[:, j, :], pb[:],
                            mybir.ActivationFunctionType.Copy,
                            scale=dis_sb[:, t : t + 1],
                        )
                nc.sync.dma_start(out_d[sci], yst[:])

            agg_layer(2, l2_out)

    nc.compile()
    return nc


# ------------------------------------------------------------------ driver
_CACHE = {}
TRACE = False
LAST_RESULTS = None


def kernel(x, edge_index, W1, b1, W2, b2):
    from concourse.bass_utils import run_bass_kernel_spmd

    x = np.asarray(x)
    edge_index = np.asarray(edge_index)
    W1 = np.asarray(W1, dtype=np.float32)
    b1 = np.asarray(b1, dtype=np.float32)
    W2 = np.asarray(W2, dtype=np.float32)
    b2 = np.asarray(b2, dtype=np.float32)

    use_prep = os.environ.get("GCN2_PREP", "0") == "1"
    b_nonzero = bool(np.any(b1) or np.any(b2))
    pp = _prep(x, edge_index)
    key = (
        x.shape, edge_index.shape, pp["NG"], use_prep, b_nonzero,
        os.environ.get("GCN2_QN", "4"),
    )
    if key not in _CACHE:
        _CACHE[key] = _build(pp, use_prep, b_nonzero)
    nc = _CACHE[key]

    b1r = np.broadcast_to(b1, (P, F_HID)).copy()
    b2r = np.broadcast_to(b2, (P, F_OUT)).copy()
    in_maps = []
    for c in range(NCORES):
        m = {
            "xg": pp["xg"][c],
            "s1t": pp["s1"][c],
            "idxt": pp["idx_sb"][c],
            "srelt": pp["srel_sb"][c],
            "srel0t": pp["srel0_sb"][c],
            "dist": pp["dis_sb"][c],
            "dis2t": pp["dis2_sb"][c],
            "iotat": pp["iota_t"],
            "iotasc": pp["iota_sc"],
            "W1h": W1.astype(np.float16),
            "b1r": b1r,
            "W2h": W2.astype(np.float16),
            "b2r": b2r,
        }
        in_maps.append(m)
    res = run_bass_kernel_spmd(
        nc, in_maps, list(range(NCORES)), trace=TRACE
    )
    global LAST_RESULTS
    LAST_RESULTS = res
    outs = np.stack(
        [np.asarray(r["out"]).reshape(pp["R"], F_OUT) for r in res.results]
    )  # [C, R, FO]
    outs = outs.reshape(NCORES * pp["R"], F_OUT)
    return np.ascontiguousarray(outs[pp["node_row"]])


# revision 22
# speedup vs baseline: 1.0527x; 1.0114x over previous
"""2-layer GCN on 8 Trainium2 NeuronCores (Bass/Tile SPMD kernel).

Math: reference computes, per layer,
    out = A_norm @ (in @ W) + b,   A_norm[d,s] = dis[d]*dis[s]*A_hat[d,s]
with A_hat = adjacency + self-loops, dis = 1/sqrt(deg).
We use associativity to aggregate first:
    out = dis ⊙ (A_hat @ (dis ⊙ in)) @ W + b
so the per-edge work is a pure gather+segment-sum of pre-scaled node
features (no per-edge multiplies).

Sharding: nodes are balanced across 8 cores x NSC superchunks of 512
"slots" each (NSC padded to a multiple of NB so AllGather quarters are
superchunk-aligned).  Edges are assigned to the core/superchunk of their
dst node, bucketed by src block (so gather indices fit in int16), sorted
by dst slot, and packed into 128-edge groups.  Each group is one one-hot
matmul G.T @ S that segment-sums the group into PSUM[feat, 512 slots].

Layer 1 consumes a HOST-pre-gathered fp8 edge-feature stream (xg) plus a
HOST-prebuilt fp8 one-hot stream (S) — both pure sequential DMA, no
on-chip gather or S-build.  Layer 2 gathers the fp16 hidden features
with pipelined dma_gather (prepare_only + per-queue trigger/sems) from
the AllGathered hidden state; its one-hot S is built on the DVE
(is_equal against an iota table) since the gathered operand is fp16.

Per-node output rows are stored "staged" (row = p*4 + j for slot
j*128+p) so each superchunk's four activation tiles land in one
contiguous [128, 4, F] DMA.

The gather-source row space is quartered: row = q*BLOCK + core*QR + o
so that each of 4 chunked AllGathers of the hidden layer delivers
exactly one gather bucket, letting layer-2 aggregation start while
later chunks are still in flight.
"""

import os
import sys

import ml_dtypes
import numpy as np

sys.path.insert(0, "/opt/trn_rl_repo")

P = 128          # partitions / group size
SC = 512         # slots per superchunk (= one PSUM bank of f32)
NCORES = 8
F_IN = 128
F_HID = 128
F_OUT = 64
NB = 4           # src buckets == AllGather chunks (int16 gather idx limit)
TPS = SC // P    # tiles per superchunk (4)

WMAX = int(os.environ.get("GCN2_WMAX", "64"))   # max slot span of a group
GCHUNK = int(os.environ.get("GCN2_L2G", "8"))  # ucode cap: 1024 idxs/call  # groups per gather call
XF8 = os.environ.get("GCN2_XF8", "1") == "1"    # fp8 layer-1 streams

F8 = ml_dtypes.float8_e4m3


def _chunks(ng):
    n = -(-ng // GCHUNK)
    base = ng // n
    rem = ng % n
    return [base + (1 if i < rem else 0) for i in range(n)]


# ----------------------------------------------------------------- host prep
def _prep(x, edge_index):
    N, F = x.shape
    assert F == F_IN
    src0 = np.asarray(edge_index[0], dtype=np.int64)
    dst0 = np.asarray(edge_index[1], dtype=np.int64)

    deg = np.bincount(dst0, minlength=N).astype(np.float32) + 1.0
    dis = (1.0 / np.sqrt(deg)).astype(np.float32)

    # --- node -> (core, superchunk, slot), balancing edge counts per bin
    NSC = int(np.ceil(N / (NCORES * SC)))
    NSC = -(-NSC // NB) * NB          # quarter-aligned superchunks
    NSCQ = NSC // NB
    nbins = NCORES * NSC
    R = NSC * SC                      # padded rows per core
    QR = R // NB                      # rows per core per quarter
    BLOCK = NCORES * QR               # rows per gather bucket
    assert BLOCK <= 32768

    order = np.argsort(-deg, kind="stable")
    k = np.arange(N)
    rnd = k // nbins                      # deal round
    col = k % nbins
    bin_of_sorted = np.where(rnd % 2 == 0, col, nbins - 1 - col)
    bin_id = np.empty(N, dtype=np.int64)
    bin_id[order] = bin_of_sorted
    pos_in_bin = np.empty(N, dtype=np.int64)
    pos_in_bin[order] = rnd

    npb = int(np.ceil(N / nbins))
    assert npb <= SC
    rng = np.random.default_rng(12345)
    perms = np.stack([rng.permutation(SC)[:npb] for _ in range(nbins)])
    slot = perms[bin_id, pos_in_bin]
    core = bin_id // NSC
    sc = bin_id % NSC
    rw = sc * SC + slot                   # core-local LOGICAL row id
    # staged row-within-superchunk: partition-major so a superchunk's 4
    # activation tiles write one contiguous [128, 4, F] block
    stg = (slot % P) * TPS + slot // P
    srow = sc * SC + stg                  # core-local STAGED row id
    node_row = core * R + srow            # padded output row id (staged)
    # quartered gather-source layout: bucket == AllGather chunk
    grow = (sc // NSCQ) * BLOCK + core * QR + (sc % NSCQ) * SC + stg

    # --- messages (edges + self loops), sorted by (cell, slot)
    loop = np.arange(N)
    ms = grow[np.concatenate([src0, loop])]
    md_core = np.concatenate([core[dst0], core[loop]])
    md_rw = np.concatenate([rw[dst0], rw[loop]])
    m_sc = md_rw // SC
    m_slot = md_rw % SC
    m_bkt = ms // BLOCK
    cell = ((md_core * NSC) + m_sc) * NB + m_bkt
    key = cell * SC + m_slot
    o = np.argsort(key, kind="stable")
    ms_s, cell_s, slot_s = ms[o], cell[o], m_slot[o]

    ncells = NCORES * NSC * NB
    cell_starts = np.searchsorted(cell_s, np.arange(ncells))
    cell_ends = np.searchsorted(cell_s, np.arange(ncells) + 1)

    # --- pack cells into groups of <=128 edges spanning < WMAX slots.
    # Window boundaries are SHARED across the 8 cores (close a window when
    # any core reaches 128 edges or the span reaches WMAX), so the PSUM
    # window offsets are compile-time constants — no per-cell register
    # loads on the PE engine.
    groups = [[] for _ in range(ncells)]   # (start, end, lo); may be empty
    lo_list = [[] for _ in range(NSC * NB)]
    for sci in range(NSC):
        for b in range(NB):
            scb = sci * NB + b
            arrs, base = [], []
            for co in range(NCORES):
                c = (co * NSC + sci) * NB + b
                s, e = int(cell_starts[c]), int(cell_ends[c])
                arrs.append(slot_s[s:e])
                base.append(s)
            ptr = [0] * NCORES
            while any(ptr[co] < len(arrs[co]) for co in range(NCORES)):
                lo = min(
                    int(arrs[co][ptr[co]])
                    for co in range(NCORES)
                    if ptr[co] < len(arrs[co])
                )
                lo = min(lo, SC - WMAX)
                hi = lo + WMAX
                for co in range(NCORES):
                    a, p0 = arrs[co], ptr[co]
                    pe_ = int(np.searchsorted(a, hi))
                    if pe_ - p0 > P:
                        hi = int(a[p0 + P])
                assert hi > lo, "slot tie overflow (>128 edges on one slot)"
                for co in range(NCORES):
                    a, p0 = arrs[co], ptr[co]
                    cnt = int(np.searchsorted(a, hi)) - p0
                    assert cnt <= P
                    c = (co * NSC + sci) * NB + b
                    groups[c].append(
                        (base[co] + p0, base[co] + p0 + cnt, lo)
                    )
                    ptr[co] += cnt
                lo_list[scb].append(lo)
    NG = max(1, max(len(g) for g in groups))

    # --- per-core tables
    # idx padding: inside/between real groups pad with 0 (gathered but
    # ignored via zero one-hot rows).  Per gather chunk, the static
    # num_idxs is the max real count across the 8 cores; beyond it idxs
    # are -1 so the DMA skips the common tail.
    chunk_sizes = _chunks(NG)
    nch = len(chunk_sizes)
    cbase = np.concatenate([[0], np.cumsum(chunk_sizes)])  # group offsets
    ncols = NSC * NB * NG
    idx_tab = np.zeros((NCORES, NSC * NB, NG * P), dtype=np.int16)
    srel_tab = np.full((NCORES, ncols, P), -1.0, dtype=np.float16)
    srel0_tab = np.full((NCORES, NSC, P), -1.0, dtype=np.float32)
    cnt_tab = np.ones((NCORES, NSC * NB, nch), dtype=np.int32)
    for c in range(ncells):
        co, rem = divmod(c, NSC * NB)
        scb = rem                    # (sc*NB + b) index
        sci, b = divmod(rem, NB)
        glist = groups[c]
        for g, (s, e, lo) in enumerate(glist):
            n = e - s
            base = scb * NG + g
            idx_tab[co, scb, g * P : g * P + n] = (ms_s[s:e] - b * BLOCK).astype(
                np.int16
            )
            if b == 0 and g == 0:
                srel0_tab[co, sci, :n] = slot_s[s:e].astype(np.float32)
            else:
                srel_tab[co, base, :n] = (slot_s[s:e] - lo).astype(np.float16)
        for ci in range(nch):
            g0, g1 = cbase[ci], cbase[ci + 1]
            cnt_tab[co, scb, ci] = max(min(len(glist), g1) - g0, 0)
    # static per-(cell, chunk) group count = max across cores (>=1 for
    # chunk 0 so the S0 start-matmul always has a gathered tile)
    gcnt = cnt_tab.max(axis=0)                 # [NSC*NB, nch] in groups
    gcnt[:, 0] = np.maximum(gcnt[:, 0], 1)

    # group offsets in consumption order (sci, b, ci) — shared by the
    # pre-gathered layer-1 feature/one-hot streams and their loads
    offs = np.zeros((NSC * NB, nch), dtype=np.int64)
    tot = 0
    for scb in range(NSC * NB):
        for ci in range(nch):
            offs[scb, ci] = tot
            tot += int(gcnt[scb, ci])
    NGTOT = tot
    # per-superchunk stream extents (layer 1 loads one block per sci)
    sci_off = np.array(
        [offs[sci * NB, 0] for sci in range(NSC)] + [NGTOT], dtype=np.int64
    )
    GSMAX = int((sci_off[1:] - sci_off[:-1]).max())

    # layer-1 edge features pre-gathered on the host, partition-major:
    # xg[p, gidx, :] = dis-scaled x of the src of edge (gidx, p), 0 if pad
    xg_rows = np.full((NCORES, NGTOT * P), -1, dtype=np.int64)
    for c in range(ncells):
        co, rem = divmod(c, NSC * NB)
        scb = rem
        for g, (s, e, lo) in enumerate(groups[c]):
            ci = int(np.searchsorted(cbase, g, "right") - 1)
            pos = offs[scb, ci] + (g - cbase[ci])
            xg_rows[co, pos * P : pos * P + (e - s)] = ms_s[s:e]

    # wrap idx to [16, cols] then replicate to 128 partitions
    idx_wrapped = idx_tab.reshape(NCORES, NSC * NB, NG * P // 16, 16)
    idx_wrapped = np.transpose(idx_wrapped, (0, 1, 3, 2))  # [.., 16, NG*8]
    idx_wrapped = np.tile(idx_wrapped, (1, 1, 8, 1))       # [.., 128, NG*8]
    # final SBUF-layout table per core: [128, NSC*NB*NG*8]
    idx_sb = np.ascontiguousarray(
        np.transpose(idx_wrapped, (0, 2, 1, 3)).reshape(NCORES, P, -1)
    )
    srel_sb = np.ascontiguousarray(np.transpose(srel_tab, (0, 2, 1)))
    srel0_sb = np.ascontiguousarray(np.transpose(srel0_tab, (0, 2, 1)))

    # per-core dis column tables [128, NT] (logical tile-major layout)
    NT = R // P
    dis_pad = np.zeros(NCORES * R, dtype=np.float32)
    dis_pad[core * R + rw] = dis
    dis_sb = np.ascontiguousarray(
        dis_pad.reshape(NCORES, NT, P).transpose(0, 2, 1)
    )
    dis2_sb = np.ascontiguousarray(dis_sb * dis_sb)

    # gather-source xs in the grow layout, pre-scaled by dis
    fdt = F8 if XF8 else np.float16
    xs_g = np.zeros((NCORES * R + 1, F_IN), dtype=fdt)
    xs_g[grow] = (x.astype(np.float32) * dis[:, None]).astype(fdt)
    # pad rows (-1) read the trailing zero row
    xg = [
        np.ascontiguousarray(
            xs_g[xg_rows[co]].reshape(NGTOT, P, F_IN).transpose(1, 0, 2)
        )
        for co in range(NCORES)
    ]

    iota_t = np.tile(np.arange(WMAX, dtype=np.float16), NG)
    iota_t = np.broadcast_to(iota_t, (P, NG * WMAX)).reshape(P, NG, WMAX).copy()
    iota_sc = np.broadcast_to(
        np.arange(SC, dtype=np.float16), (P, SC)
    ).copy()

    return dict(
        N=N, NSC=NSC, NSCQ=NSCQ, R=R, QR=QR, BLOCK=BLOCK, NG=NG, NT=NT,
        node_row=node_row, xg=xg, NGTOT=NGTOT, offs=offs,
        sci_off=sci_off, GSMAX=GSMAX,
        idx_sb=idx_sb, srel_sb=srel_sb, srel0_sb=srel0_sb,
        lo_list=lo_list, gcnt=gcnt,
        dis_sb=dis_sb, dis2_sb=dis2_sb, iota_t=iota_t, iota_sc=iota_sc,
    )


# ------------------------------------------------------------- bass program
def _build(pp, use_prep, b_nonzero):
    import concourse.bass as bass
    import concourse.bacc as bacc
    import concourse.mybir as mybir
    from concourse import tile

    f32 = mybir.dt.float32
    f16 = mybir.dt.float16
    f8 = mybir.dt.float8e4
    i16 = mybir.dt.int16
    fdat1 = f8 if XF8 else f16
    NSC, NSCQ, R, QR, BLOCK = pp["NSC"], pp["NSCQ"], pp["R"], pp["QR"], pp["BLOCK"]
    NG, NT = pp["NG"], pp["NT"]
    ncols = NSC * NB * NG
    chunk_sizes = _chunks(NG)
    gmax = max(chunk_sizes)
    v_g1bufs = int(os.environ.get("GCN2_G1BUFS", "3"))
    v_g2bufs = int(os.environ.get("GCN2_G2BUFS", "16"))

    scratch = int(os.environ.get("GCN2_SCRATCH", str(64 * 1024)))
    v_qn = int(os.environ.get("GCN2_QN", "4"))
    nc = bacc.Bacc(
        "TRN2", target_bir_lowering=False, debug=False, num_devices=NCORES,
        dynamic_dma_scratch_size=scratch, num_swdge_queues=v_qn,
    )

    NGTOT = pp["NGTOT"]
    offs = pp["offs"]
    sci_off = pp["sci_off"]
    GSMAX = pp["GSMAX"]
    xg_d = nc.dram_tensor("xg", [P, NGTOT, F_IN], fdat1, kind="ExternalInput")
    idx_d = nc.dram_tensor("idxt", [P, ncols * 8], i16, kind="ExternalInput")
    srel_d = nc.dram_tensor("srelt", [P, ncols], f16, kind="ExternalInput")
    srel0_d = nc.dram_tensor("srel0t", [P, NSC], f32, kind="ExternalInput")
    lo_list = pp["lo_list"]
    nch = len(chunk_sizes)
    gcnt = pp["gcnt"]
    cbase = [0]
    for gn in chunk_sizes:
        cbase.append(cbase[-1] + gn)
    dis_d = nc.dram_tensor("dist", [P, NT], f32, kind="ExternalInput")
    dis2_d = nc.dram_tensor("dis2t", [P, NT], f32, kind="ExternalInput")
    it_d = nc.dram_tensor("iotat", [P, NG, WMAX], f16, kind="ExternalInput")
    isc_d = nc.dram_tensor("iotasc", [P, SC], f16, kind="ExternalInput")
    W1_d = nc.dram_tensor("W1h", [F_IN, F_HID], f16, kind="ExternalInput")
    b1_d = nc.dram_tensor("b1r", [P, F_HID], f32, kind="ExternalInput")
    W2_d = nc.dram_tensor("W2h", [F_HID, F_OUT], f16, kind="ExternalInput")
    b2_d = nc.dram_tensor("b2r", [P, F_OUT], f32, kind="ExternalInput")
    out_d = nc.dram_tensor("out", [NSC, P, TPS, F_OUT], f32, kind="ExternalOutput")

    u2loc = [
        nc.dram_tensor(f"u2loc{q}", [NSCQ, P, TPS, F_HID], f16) for q in range(NB)
    ]
    u2g = [
        nc.dram_tensor(f"u2g{q}", [BLOCK, F_HID], f16, addr_space="Shared")
        for q in range(NB)
    ]

    dma_sems = [nc.alloc_semaphore(f"gsem{q}") for q in range(v_qn)]

    with tile.TileContext(nc) as tc:
        with (
            tc.tile_pool(name="const", bufs=1) as cpool,
            tc.tile_pool(name="vt", bufs=3) as vpool,
            tc.tile_pool(name="g1", bufs=v_g1bufs) as g1pool,
            tc.tile_pool(name="g2", bufs=v_g2bufs) as g2pool,
            tc.tile_pool(name="smat", bufs=4) as s2pool,
            tc.tile_pool(name="s0mat", bufs=2) as s0pool,
            tc.tile_pool(name="uwork", bufs=3) as upool,
            tc.tile_pool(name="psagg", bufs=4, space="PSUM") as pagg,
            tc.tile_pool(name="psmm", bufs=2, space="PSUM") as pmm,
        ):
            # ---- constants / tables resident in SBUF
            idx_sb = cpool.tile([P, ncols * 8], i16)
            srel_sb = cpool.tile([P, ncols], f16)
            srel0_sb = cpool.tile([P, NSC], f32)
            dis_sb = cpool.tile([P, NT], f32)
            dis2_sb = cpool.tile([P, NT], f32)
            it_sb = cpool.tile([P, NG, WMAX], f16)
            isc_sb = cpool.tile([P, SC], f16)
            W1_sb = cpool.tile([F_IN, F_HID], f16)
            b1_sb = cpool.tile([P, F_HID], f32)
            W2_sb = cpool.tile([F_HID, F_OUT], f16)
            b2_sb = cpool.tile([P, F_OUT], f32)
            for sb, d in [
                (idx_sb, idx_d), (srel_sb, srel_d), (srel0_sb, srel0_d),
                (dis_sb, dis_d), (dis2_sb, dis2_d), (isc_sb, isc_d),
                (W1_sb, W1_d), (b1_sb, b1_d), (W2_sb, W2_d), (b2_sb, b2_d),
                (it_sb, it_d),
            ]:
                nc.sync.dma_start(sb[:], d[:])

            qctr = [0]

            def load_l2(scb, ci, ge):
                b = scb % NB
                g0 = cbase[ci]
                q = qctr[0] % v_qn
                qctr[0] += 1
                gt = g2pool.tile([P, gmax, F_IN], f16, tag="g2")
                args = dict(elem_step=F_IN, queue_num=q)
                if use_prep:
                    args.update(prepare_only=True, sem=dma_sems[q])
                nc.gpsimd.dma_gather(
                    gt[:, :ge, :],
                    u2g[b][:],
                    idx_sb[:, (scb * NG + g0) * 8 : (scb * NG + g0 + ge) * 8],
                    ge * P, ge * P, F_IN, **args,
                )
                if use_prep:
                    nc.gpsimd.trigger_dma(count=None, queue_num=q)
                return gt

            def agg_layer(layer, out_cb):
                """out_cb(sci, ps) with ps = (A_hat @ src)^T for superchunk."""
                s0dt = fdat1 if layer == 1 else f16
                for sci in range(NSC):
                    ps = pagg.tile([P, SC], f32)
                    ngrp = sum(
                        int(gcnt[sci * NB + b, ci])
                        for b in range(NB)
                        for ci in range(nch)
                    )
                    gi = 0
                    if layer == 1:
                        # one bulk load pair for the whole superchunk
                        off0 = int(sci_off[sci])
                        gs = int(sci_off[sci + 1]) - off0
                        gt1 = g1pool.tile([P, GSMAX, F_IN], fdat1, tag="g1")
                        nc.sync.dma_start(
                            gt1[:, :gs, :], xg_d[:, off0 : off0 + gs, :]
                        )
                    for b in range(NB):
                        scb = sci * NB + b
                        tiles = {}
                        if layer == 2:
                            for ci in range(nch):
                                ge = int(gcnt[scb, ci])
                                if ge == 0:
                                    continue
                                tiles[ci] = load_l2(scb, ci, ge)
                        # one-hot matrices for the whole cell in one op
                        # (fp8 for layer 1 to match the fp8 edge stream)
                        S2 = s2pool.tile(
                            [P, NG, WMAX], s0dt, tag=f"s_l{layer}"
                        )
                        nc.vector.tensor_tensor(
                            S2[:],
                            it_sb[:],
                            srel_sb[:, scb * NG : (scb + 1) * NG]
                            .unsqueeze(2)
                            .broadcast_to((P, NG, WMAX)),
                            op=mybir.AluOpType.is_equal,
                        )
                        if b == 0:
                            S0 = s0pool.tile([P, SC], s0dt, tag=f"s0l{layer}")
                            nc.vector.tensor_scalar(
                                S0[:],
                                isc_sb[:],
                                srel0_sb[:, sci : sci + 1],
                                None,
                                op0=mybir.AluOpType.is_equal,
                            )
                        for ci in range(nch):
                            ge = int(gcnt[scb, ci])
                            for gg in range(ge):
                                g = cbase[ci] + gg
                                if layer == 1:
                                    pos = int(offs[scb, ci]) + gg - off0
                                    lhs = gt1[:, pos, :]
                                else:
                                    lhs = tiles[ci][:, gg, :]
                                rhs = S2[:, g, :]
                                if b == 0 and g == 0:
                                    nc.tensor.matmul(
                                        ps[:, :],
                                        lhs,
                                        S0[:],
                                        start=True,
                                        stop=(gi == ngrp - 1),
                                    )
                                else:
                                    lo = lo_list[scb][g]
                                    nc.tensor.matmul(
                                        ps[:, lo : lo + WMAX],
                                        lhs,
                                        rhs,
                                        start=False,
                                        stop=(gi == ngrp - 1),
                                    )
                                gi += 1
                    out_cb(sci, ps)

            # ---------------- layer 1
            def l1_out(sci, ps):
                v = vpool.tile([P, SC], f16, tag="v")
                nc.scalar.copy(v[:], ps[:])
                ust = upool.tile([P, TPS, F_HID], f16, tag="u")
                for j in range(TPS):
                    t = sci * TPS + j
                    pb = pmm.tile([P, F_HID], f32, tag="pb")
                    nc.tensor.matmul(
                        pb[:], v[:, j * P : (j + 1) * P], W1_sb[:],
                        start=True, stop=True,
                    )
                    if b_nonzero:
                        w = upool.tile([P, F_HID], f32, tag="w")
                        nc.vector.tensor_scalar(
                            w[:], pb[:], dis_sb[:, t : t + 1], None,
                            op0=mybir.AluOpType.mult,
                        )
                        nc.vector.tensor_tensor(
                            w[:], w[:], b1_sb[:], op=mybir.AluOpType.add
                        )
                        nc.scalar.activation(
                            ust[:, j, :], w[:],
                            mybir.ActivationFunctionType.Relu,
                            scale=dis_sb[:, t : t + 1],
                        )
                    else:
                        # u = dis * relu(dis*agg@W1) = relu(dis^2 * agg@W1)
                        nc.scalar.activation(
                            ust[:, j, :], pb[:],
                            mybir.ActivationFunctionType.Relu,
                            scale=dis2_sb[:, t : t + 1],
                        )
                q, scq = divmod(sci, NSCQ)
                nc.sync.dma_start(u2loc[q][scq], ust[:])
                if scq == NSCQ - 1:
                    nc.gpsimd.collective_compute(
                        "AllGather",
                        mybir.AluOpType.bypass,
                        replica_groups=[list(range(NCORES))],
                        ins=[u2loc[q][:]],
                        outs=[u2g[q][:]],
                    )

            agg_layer(1, l1_out)

            # ---------------- layer 2
            def l2_out(sci, ps):
                v = vpool.tile([P, SC], f16, tag="v")
                nc.scalar.copy(v[:], ps[:])
                yst = upool.tile([P, TPS, F_OUT], f32, tag="y")
                for j in range(TPS):
                    t = sci * TPS + j
                    pb = pmm.tile([P, F_OUT], f32, tag="pe")
                    nc.tensor.matmul(
                        pb[:], v[:, j * P : (j + 1) * P], W2_sb[:],
                        start=True, stop=True,
                    )
                    if b_nonzero:
                        y = upool.tile([P, F_OUT], f32, tag="yb")
                        nc.vector.tensor_scalar(
                            y[:], pb[:], dis_sb[:, t : t + 1], None,
                            op0=mybir.AluOpType.mult,
                        )
                        nc.vector.tensor_tensor(
                            yst[:, j, :], y[:], b2_sb[:],
                            op=mybir.AluOpType.add,
                        )
                    else:
                        nc.scalar.activation(
                            yst[:, j, :], pb[:],
                            mybir.ActivationFunctionType.Copy,
                            scale=dis_sb[:, t : t + 1],
                        )
                nc.sync.dma_start(out_d[sci], yst[:])

            agg_layer(2, l2_out)

    nc.compile()
    return nc


# ------------------------------------------------------------------ driver
_CACHE = {}
TRACE = False
LAST_RESULTS = None


def kernel(x, edge_index, W1, b1, W2, b2):
    from concourse.bass_utils import run_bass_kernel_spmd

    x = np.asarray(x)
    edge_index = np.asarray(edge_index)
    W1 = np.asarray(W1, dtype=np.float32)
    b1 = np.asarray(b1, dtype=np.float32)
    W2 = np.asarray(W2, dtype=np.float32)
    b2 = np.asarray(b2, dtype=np.float32)

    use_prep = os.environ.get("GCN2_PREP", "0") == "1"
    b_nonzero = bool(np.any(b1) or np.any(b2))
    pp = _prep(x, edge_index)
    key = (
        x.shape, edge_index.shape, pp["NG"], use_prep, b_nonzero,
        os.environ.get("GCN2_QN", "4"),
    )
    if key not in _CACHE:
        _CACHE[key] = _build(pp, use_prep, b_nonzero)
    nc = _CACHE[key]

    b1r = np.broadcast_to(b1, (P, F_HID)).copy()
    b2r = np.broadcast_to(b2, (P, F_OUT)).copy()
    in_maps = []
    for c in range(NCORES):
        m = {
            "xg": pp["xg"][c],
            "idxt": pp["idx_sb"][c],
            "srelt": pp["srel_sb"][c],
            "srel0t": pp["srel0_sb"][c],
            "dist": pp["dis_sb"][c],
            "dis2t": pp["dis2_sb"][c],
            "iotat": pp["iota_t"],
            "iotasc": pp["iota_sc"],
            "W1h": W1.astype(np.float16),
            "b1r": b1r,
            "W2h": W2.astype(np.float16),
            "b2r": b2r,
        }
        in_maps.append(m)
    res = run_bass_kernel_spmd(
        nc, in_maps, list(range(NCORES)), trace=TRACE
    )
    global LAST_RESULTS
    LAST_RESULTS = res
    outs = np.stack(
        [np.asarray(r["out"]).reshape(pp["R"], F_OUT) for r in res.results]
    )  # [C, R, FO]
    outs = outs.reshape(NCORES * pp["R"], F_OUT)
    return np.ascontiguousarray(outs[pp["node_row"]])


# revision 24
# speedup vs baseline: 1.0706x; 1.0171x over previous
"""2-layer GCN on 8 Trainium2 NeuronCores (Bass/Tile SPMD kernel).

Math: reference computes, per layer,
    out = A_norm @ (in @ W) + b,   A_norm[d,s] = dis[d]*dis[s]*A_hat[d,s]
with A_hat = adjacency + self-loops, dis = 1/sqrt(deg).
We use associativity to aggregate first:
    out = dis ⊙ (A_hat @ (dis ⊙ in)) @ W + b
so the per-edge work is a pure gather+segment-sum of pre-scaled node
features (no per-edge multiplies).

Sharding: nodes are balanced across 8 cores x NSC superchunks of 512
"slots" each (NSC padded to a multiple of NB so AllGather quarters are
superchunk-aligned).  Edges are assigned to the core/superchunk of their
dst node, bucketed by src block (so gather indices fit in int16), sorted
by dst slot, and packed into 128-edge groups.  Each group is one one-hot
matmul G.T @ S that segment-sums the group into PSUM[feat, 512 slots].

Layer 1 consumes a HOST-pre-gathered fp8 edge-feature stream (xg) plus a
HOST-prebuilt fp8 one-hot stream (S) — both pure sequential DMA, no
on-chip gather or S-build.  Layer 2 gathers the fp16 hidden features
with pipelined dma_gather (prepare_only + per-queue trigger/sems) from
the AllGathered hidden state; its one-hot S is built on the DVE
(is_equal against an iota table) since the gathered operand is fp16.

Per-node output rows are stored "staged" (row = p*4 + j for slot
j*128+p) so each superchunk's four activation tiles land in one
contiguous [128, 4, F] DMA.

The gather-source row space is quartered: row = q*BLOCK + core*QR + o
so that each of 4 chunked AllGathers of the hidden layer delivers
exactly one gather bucket, letting layer-2 aggregation start while
later chunks are still in flight.
"""

import os
import sys

import ml_dtypes
import numpy as np

sys.path.insert(0, "/opt/trn_rl_repo")

P = 128          # partitions / group size
SC = 512         # slots per superchunk (= one PSUM bank of f32)
NCORES = 8
F_IN = 128
F_HID = 128
F_OUT = 64
NB = 4           # src buckets == AllGather chunks (int16 gather idx limit)
TPS = SC // P    # tiles per superchunk (4)

WMAX = int(os.environ.get("GCN2_WMAX", "64"))   # max slot span of a group
GCHUNK = int(os.environ.get("GCN2_L2G", "8"))  # ucode cap: 1024 idxs/call  # groups per gather call
XF8 = os.environ.get("GCN2_XF8", "1") == "1"    # fp8 layer-1 streams

F8 = ml_dtypes.float8_e4m3


def _chunks(ng):
    n = -(-ng // GCHUNK)
    base = ng // n
    rem = ng % n
    return [base + (1 if i < rem else 0) for i in range(n)]


# ----------------------------------------------------------------- host prep
def _prep(x, edge_index):
    N, F = x.shape
    assert F == F_IN
    src0 = np.asarray(edge_index[0], dtype=np.int64)
    dst0 = np.asarray(edge_index[1], dtype=np.int64)

    deg = np.bincount(dst0, minlength=N).astype(np.float32) + 1.0
    dis = (1.0 / np.sqrt(deg)).astype(np.float32)

    # --- node -> (core, superchunk, slot), balancing edge counts per bin
    NSC = int(np.ceil(N / (NCORES * SC)))
    NSC = -(-NSC // NB) * NB          # quarter-aligned superchunks
    NSCQ = NSC // NB
    nbins = NCORES * NSC
    R = NSC * SC                      # padded rows per core
    QR = R // NB                      # rows per core per quarter
    BLOCK = NCORES * QR               # rows per gather bucket
    assert BLOCK <= 32768

    order = np.argsort(-deg, kind="stable")
    k = np.arange(N)
    rnd = k // nbins                      # deal round
    col = k % nbins
    bin_of_sorted = np.where(rnd % 2 == 0, col, nbins - 1 - col)
    bin_id = np.empty(N, dtype=np.int64)
    bin_id[order] = bin_of_sorted
    pos_in_bin = np.empty(N, dtype=np.int64)
    pos_in_bin[order] = rnd

    npb = int(np.ceil(N / nbins))
    assert npb <= SC
    rng = np.random.default_rng(12345)
    perms = np.stack([rng.permutation(SC)[:npb] for _ in range(nbins)])
    slot = perms[bin_id, pos_in_bin]
    core = bin_id // NSC
    sc = bin_id % NSC
    rw = sc * SC + slot                   # core-local LOGICAL row id
    # staged row-within-superchunk: partition-major so a superchunk's 4
    # activation tiles write one contiguous [128, 4, F] block
    stg = (slot % P) * TPS + slot // P
    srow = sc * SC + stg                  # core-local STAGED row id
    node_row = core * R + srow            # padded output row id (staged)
    # quartered gather-source layout: bucket == AllGather chunk
    grow = (sc // NSCQ) * BLOCK + core * QR + (sc % NSCQ) * SC + stg

    # --- messages (edges + self loops), sorted by (cell, slot)
    loop = np.arange(N)
    ms = grow[np.concatenate([src0, loop])]
    md_core = np.concatenate([core[dst0], core[loop]])
    md_rw = np.concatenate([rw[dst0], rw[loop]])
    m_sc = md_rw // SC
    m_slot = md_rw % SC
    m_bkt = ms // BLOCK
    cell = ((md_core * NSC) + m_sc) * NB + m_bkt
    key = cell * SC + m_slot
    o = np.argsort(key, kind="stable")
    ms_s, cell_s, slot_s = ms[o], cell[o], m_slot[o]

    ncells = NCORES * NSC * NB
    cell_starts = np.searchsorted(cell_s, np.arange(ncells))
    cell_ends = np.searchsorted(cell_s, np.arange(ncells) + 1)

    # --- pack cells into groups of <=128 edges spanning < WMAX slots.
    # Window boundaries are SHARED across the 8 cores (close a window when
    # any core reaches 128 edges or the span reaches WMAX), so the PSUM
    # window offsets are compile-time constants — no per-cell register
    # loads on the PE engine.
    groups = [[] for _ in range(ncells)]   # (start, end, lo); may be empty
    lo_list = [[] for _ in range(NSC * NB)]
    for sci in range(NSC):
        for b in range(NB):
            scb = sci * NB + b
            arrs, base = [], []
            for co in range(NCORES):
                c = (co * NSC + sci) * NB + b
                s, e = int(cell_starts[c]), int(cell_ends[c])
                arrs.append(slot_s[s:e])
                base.append(s)
            ptr = [0] * NCORES
            while any(ptr[co] < len(arrs[co]) for co in range(NCORES)):
                lo = min(
                    int(arrs[co][ptr[co]])
                    for co in range(NCORES)
                    if ptr[co] < len(arrs[co])
                )
                lo = min(lo, SC - WMAX)
                hi = lo + WMAX
                for co in range(NCORES):
                    a, p0 = arrs[co], ptr[co]
                    pe_ = int(np.searchsorted(a, hi))
                    if pe_ - p0 > P:
                        hi = int(a[p0 + P])
                assert hi > lo, "slot tie overflow (>128 edges on one slot)"
                for co in range(NCORES):
                    a, p0 = arrs[co], ptr[co]
                    cnt = int(np.searchsorted(a, hi)) - p0
                    assert cnt <= P
                    c = (co * NSC + sci) * NB + b
                    groups[c].append(
                        (base[co] + p0, base[co] + p0 + cnt, lo)
                    )
                    ptr[co] += cnt
                lo_list[scb].append(lo)
    NG = max(1, max(len(g) for g in groups))

    # --- per-core tables
    # idx padding: inside/between real groups pad with 0 (gathered but
    # ignored via zero one-hot rows).  Per gather chunk, the static
    # num_idxs is the max real count across the 8 cores; beyond it idxs
    # are -1 so the DMA skips the common tail.
    chunk_sizes = _chunks(NG)
    nch = len(chunk_sizes)
    cbase = np.concatenate([[0], np.cumsum(chunk_sizes)])  # group offsets
    ncols = NSC * NB * NG
    idx_tab = np.zeros((NCORES, NSC * NB, NG * P), dtype=np.int16)
    srel_tab = np.full((NCORES, ncols, P), -1.0, dtype=np.float16)
    srel0_tab = np.full((NCORES, NSC, P), -1.0, dtype=np.float32)
    cnt_tab = np.ones((NCORES, NSC * NB, nch), dtype=np.int32)
    for c in range(ncells):
        co, rem = divmod(c, NSC * NB)
        scb = rem                    # (sc*NB + b) index
        sci, b = divmod(rem, NB)
        glist = groups[c]
        for g, (s, e, lo) in enumerate(glist):
            n = e - s
            base = scb * NG + g
            idx_tab[co, scb, g * P : g * P + n] = (ms_s[s:e] - b * BLOCK).astype(
                np.int16
            )
            if b == 0 and g == 0:
                srel0_tab[co, sci, :n] = slot_s[s:e].astype(np.float32)
            else:
                srel_tab[co, base, :n] = (slot_s[s:e] - lo).astype(np.float16)
        for ci in range(nch):
            g0, g1 = cbase[ci], cbase[ci + 1]
            cnt_tab[co, scb, ci] = max(min(len(glist), g1) - g0, 0)
    # static per-(cell, chunk) group count = max across cores (>=1 for
    # chunk 0 so the S0 start-matmul always has a gathered tile)
    gcnt = cnt_tab.max(axis=0)                 # [NSC*NB, nch] in groups
    gcnt[:, 0] = np.maximum(gcnt[:, 0], 1)

    # group offsets in consumption order (sci, b, ci) — shared by the
    # pre-gathered layer-1 feature/one-hot streams and their loads
    offs = np.zeros((NSC * NB, nch), dtype=np.int64)
    tot = 0
    for scb in range(NSC * NB):
        for ci in range(nch):
            offs[scb, ci] = tot
            tot += int(gcnt[scb, ci])
    NGTOT = tot
    # per-superchunk stream extents (layer 1 loads one block per sci)
    sci_off = np.array(
        [offs[sci * NB, 0] for sci in range(NSC)] + [NGTOT], dtype=np.int64
    )
    GSMAX = int((sci_off[1:] - sci_off[:-1]).max())

    # layer-1 edge features pre-gathered on the host, partition-major:
    # xg[p, gidx, :] = dis-scaled x of the src of edge (gidx, p), 0 if pad
    xg_rows = np.full((NCORES, NGTOT * P), -1, dtype=np.int64)
    for c in range(ncells):
        co, rem = divmod(c, NSC * NB)
        scb = rem
        for g, (s, e, lo) in enumerate(groups[c]):
            ci = int(np.searchsorted(cbase, g, "right") - 1)
            pos = offs[scb, ci] + (g - cbase[ci])
            xg_rows[co, pos * P : pos * P + (e - s)] = ms_s[s:e]

    # wrap idx to [16, cols] then replicate to 128 partitions
    idx_wrapped = idx_tab.reshape(NCORES, NSC * NB, NG * P // 16, 16)
    idx_wrapped = np.transpose(idx_wrapped, (0, 1, 3, 2))  # [.., 16, NG*8]
    idx_wrapped = np.tile(idx_wrapped, (1, 1, 8, 1))       # [.., 128, NG*8]
    # final SBUF-layout table per core: [128, NSC*NB*NG*8]
    idx_sb = np.ascontiguousarray(
        np.transpose(idx_wrapped, (0, 2, 1, 3)).reshape(NCORES, P, -1)
    )
    srel_sb = np.ascontiguousarray(np.transpose(srel_tab, (0, 2, 1)))
    srel0_sb = np.ascontiguousarray(np.transpose(srel0_tab, (0, 2, 1)))

    # per-core dis column tables [128, NT] (logical tile-major layout)
    NT = R // P
    dis_pad = np.zeros(NCORES * R, dtype=np.float32)
    dis_pad[core * R + rw] = dis
    dis_sb = np.ascontiguousarray(
        dis_pad.reshape(NCORES, NT, P).transpose(0, 2, 1)
    )
    dis2_sb = np.ascontiguousarray(dis_sb * dis_sb)

    # gather-source xs in the grow layout, pre-scaled by dis
    fdt = F8 if XF8 else np.float16
    xs_g = np.zeros((NCORES * R + 1, F_IN), dtype=fdt)
    xs_g[grow] = (x.astype(np.float32) * dis[:, None]).astype(fdt)
    # pad rows (-1) read the trailing zero row
    xg = [
        np.ascontiguousarray(
            xs_g[xg_rows[co]].reshape(NGTOT, P, F_IN).transpose(1, 0, 2)
        )
        for co in range(NCORES)
    ]

    iota_t = np.tile(np.arange(WMAX, dtype=np.float16), NG)
    iota_t = np.broadcast_to(iota_t, (P, NG * WMAX)).reshape(P, NG, WMAX).copy()
    iota_sc = np.broadcast_to(
        np.arange(SC, dtype=np.float16), (P, SC)
    ).copy()

    return dict(
        N=N, NSC=NSC, NSCQ=NSCQ, R=R, QR=QR, BLOCK=BLOCK, NG=NG, NT=NT,
        node_row=node_row, xg=xg, NGTOT=NGTOT, offs=offs,
        sci_off=sci_off, GSMAX=GSMAX,
        idx_sb=idx_sb, srel_sb=srel_sb, srel0_sb=srel0_sb,
        lo_list=lo_list, gcnt=gcnt,
        dis_sb=dis_sb, dis2_sb=dis2_sb, iota_t=iota_t, iota_sc=iota_sc,
    )


# ------------------------------------------------------------- bass program
def _build(pp, use_prep, b_nonzero):
    import concourse.bass as bass
    import concourse.bacc as bacc
    import concourse.mybir as mybir
    from concourse import tile

    f32 = mybir.dt.float32
    f16 = mybir.dt.float16
    f8 = mybir.dt.float8e4
    i16 = mybir.dt.int16
    fdat1 = f8 if XF8 else f16
    NSC, NSCQ, R, QR, BLOCK = pp["NSC"], pp["NSCQ"], pp["R"], pp["QR"], pp["BLOCK"]
    NG, NT = pp["NG"], pp["NT"]
    ncols = NSC * NB * NG
    chunk_sizes = _chunks(NG)
    gmax = max(chunk_sizes)
    v_g1bufs = int(os.environ.get("GCN2_G1BUFS", "3"))
    v_g2bufs = int(os.environ.get("GCN2_G2BUFS", "24"))

    scratch = int(os.environ.get("GCN2_SCRATCH", str(64 * 1024)))
    v_qn = int(os.environ.get("GCN2_QN", "4"))
    nc = bacc.Bacc(
        "TRN2", target_bir_lowering=False, debug=False, num_devices=NCORES,
        dynamic_dma_scratch_size=scratch, num_swdge_queues=v_qn,
    )

    NGTOT = pp["NGTOT"]
    offs = pp["offs"]
    sci_off = pp["sci_off"]
    GSMAX = pp["GSMAX"]
    xg_d = nc.dram_tensor("xg", [P, NGTOT, F_IN], fdat1, kind="ExternalInput")
    idx_d = nc.dram_tensor("idxt", [P, ncols * 8], i16, kind="ExternalInput")
    srel_d = nc.dram_tensor("srelt", [P, ncols], f16, kind="ExternalInput")
    srel0_d = nc.dram_tensor("srel0t", [P, NSC], f32, kind="ExternalInput")
    lo_list = pp["lo_list"]
    nch = len(chunk_sizes)
    gcnt = pp["gcnt"]
    cbase = [0]
    for gn in chunk_sizes:
        cbase.append(cbase[-1] + gn)
    dis_d = nc.dram_tensor("dist", [P, NT], f32, kind="ExternalInput")
    dis2_d = nc.dram_tensor("dis2t", [P, NT], f32, kind="ExternalInput")
    it_d = nc.dram_tensor("iotat", [P, NG, WMAX], f16, kind="ExternalInput")
    isc_d = nc.dram_tensor("iotasc", [P, SC], f16, kind="ExternalInput")
    W1_d = nc.dram_tensor("W1h", [F_IN, F_HID], f16, kind="ExternalInput")
    b1_d = nc.dram_tensor("b1r", [P, F_HID], f32, kind="ExternalInput")
    W2_d = nc.dram_tensor("W2h", [F_HID, F_OUT], f16, kind="ExternalInput")
    b2_d = nc.dram_tensor("b2r", [P, F_OUT], f32, kind="ExternalInput")
    out_d = nc.dram_tensor("out", [NSC, P, TPS, F_OUT], f32, kind="ExternalOutput")

    u2loc = [
        nc.dram_tensor(f"u2loc{q}", [NSCQ, P, TPS, F_HID], f16) for q in range(NB)
    ]
    u2g = [
        nc.dram_tensor(f"u2g{q}", [BLOCK, F_HID], f16, addr_space="Shared")
        for q in range(NB)
    ]

    dma_sems = [nc.alloc_semaphore(f"gsem{q}") for q in range(v_qn)]

    with tile.TileContext(nc) as tc:
        with (
            tc.tile_pool(name="const", bufs=1) as cpool,
            tc.tile_pool(name="vt", bufs=3) as vpool,
            tc.tile_pool(name="g1", bufs=v_g1bufs) as g1pool,
            tc.tile_pool(name="g2", bufs=v_g2bufs) as g2pool,
            tc.tile_pool(name="smat", bufs=8) as s2pool,
            tc.tile_pool(name="s0mat", bufs=2) as s0pool,
            tc.tile_pool(name="uwork", bufs=3) as upool,
            tc.tile_pool(name="psagg", bufs=4, space="PSUM") as pagg,
            tc.tile_pool(name="psmm", bufs=2, space="PSUM") as pmm,
        ):
            # ---- constants / tables resident in SBUF
            idx_sb = cpool.tile([P, ncols * 8], i16)
            srel_sb = cpool.tile([P, ncols], f16)
            srel0_sb = cpool.tile([P, NSC], f32)
            dis_sb = cpool.tile([P, NT], f32)
            dis2_sb = cpool.tile([P, NT], f32)
            it_sb = cpool.tile([P, NG, WMAX], f16)
            isc_sb = cpool.tile([P, SC], f16)
            W1_sb = cpool.tile([F_IN, F_HID], f16)
            b1_sb = cpool.tile([P, F_HID], f32)
            W2_sb = cpool.tile([F_HID, F_OUT], f16)
            b2_sb = cpool.tile([P, F_OUT], f32)
            for sb, d in [
                (idx_sb, idx_d), (srel_sb, srel_d), (srel0_sb, srel0_d),
                (dis_sb, dis_d), (dis2_sb, dis2_d), (isc_sb, isc_d),
                (W1_sb, W1_d), (b1_sb, b1_d), (W2_sb, W2_d), (b2_sb, b2_d),
                (it_sb, it_d),
            ]:
                nc.sync.dma_start(sb[:], d[:])

            qctr = [0]

            def load_l2(scb, ci, ge):
                b = scb % NB
                g0 = cbase[ci]
                q = qctr[0] % v_qn
                qctr[0] += 1
                gt = g2pool.tile([P, gmax, F_IN], f16, tag="g2")
                args = dict(elem_step=F_IN, queue_num=q)
                if use_prep:
                    args.update(prepare_only=True, sem=dma_sems[q])
                nc.gpsimd.dma_gather(
                    gt[:, :ge, :],
                    u2g[b][:],
                    idx_sb[:, (scb * NG + g0) * 8 : (scb * NG + g0 + ge) * 8],
                    ge * P, ge * P, F_IN, **args,
                )
                if use_prep:
                    nc.gpsimd.trigger_dma(count=None, queue_num=q)
                return gt

            def agg_layer(layer, out_cb):
                """out_cb(sci, ps) with ps = (A_hat @ src)^T for superchunk."""
                s0dt = fdat1 if layer == 1 else f16
                for sci in range(NSC):
                    ps = pagg.tile([P, SC], f32)
                    ngrp = sum(
                        int(gcnt[sci * NB + b, ci])
                        for b in range(NB)
                        for ci in range(nch)
                    )
                    gi = 0
                    if layer == 1:
                        # one bulk load pair for the whole superchunk
                        off0 = int(sci_off[sci])
                        gs = int(sci_off[sci + 1]) - off0
                        gt1 = g1pool.tile([P, GSMAX, F_IN], fdat1, tag="g1")
                        nc.sync.dma_start(
                            gt1[:, :gs, :], xg_d[:, off0 : off0 + gs, :]
                        )
                    for b in range(NB):
                        scb = sci * NB + b
                        tiles = {}
                        if layer == 2:
                            for ci in range(nch):
                                ge = int(gcnt[scb, ci])
                                if ge == 0:
                                    continue
                                tiles[ci] = load_l2(scb, ci, ge)
                        # one-hot matrices for the whole cell in one op
                        # (fp8 for layer 1 to match the fp8 edge stream)
                        S2 = s2pool.tile(
                            [P, NG, WMAX], s0dt, tag=f"s_l{layer}"
                        )
                        nc.vector.tensor_tensor(
                            S2[:],
                            it_sb[:],
                            srel_sb[:, scb * NG : (scb + 1) * NG]
                            .unsqueeze(2)
                            .broadcast_to((P, NG, WMAX)),
                            op=mybir.AluOpType.is_equal,
                        )
                        if b == 0:
                            S0 = s0pool.tile([P, SC], s0dt, tag=f"s0l{layer}")
                            nc.vector.tensor_scalar(
                                S0[:],
                                isc_sb[:],
                                srel0_sb[:, sci : sci + 1],
                                None,
                                op0=mybir.AluOpType.is_equal,
                            )
                        for ci in range(nch):
                            ge = int(gcnt[scb, ci])
                            for gg in range(ge):
                                g = cbase[ci] + gg
                                if layer == 1:
                                    pos = int(offs[scb, ci]) + gg - off0
                                    lhs = gt1[:, pos, :]
                                else:
                                    lhs = tiles[ci][:, gg, :]
                                rhs = S2[:, g, :]
                                if b == 0 and g == 0:
                                    nc.tensor.matmul(
                                        ps[:, :],
                                        lhs,
                                        S0[:],
                                        start=True,
                                        stop=(gi == ngrp - 1),
                                    )
                                else:
                                    lo = lo_list[scb][g]
                                    nc.tensor.matmul(
                                        ps[:, lo : lo + WMAX],
                                        lhs,
                                        rhs,
                                        start=False,
                                        stop=(gi == ngrp - 1),
                                    )
                                gi += 1
                    out_cb(sci, ps)

            # ---------------- layer 1
            def l1_out(sci, ps):
                v = vpool.tile([P, SC], f16, tag="v")
                nc.scalar.copy(v[:], ps[:])
                ust = upool.tile([P, TPS, F_HID], f16, tag="u")
                for j in range(TPS):
                    t = sci * TPS + j
                    pb = pmm.tile([P, F_HID], f32, tag="pb")
                    nc.tensor.matmul(
                        pb[:], v[:, j * P : (j + 1) * P], W1_sb[:],
                        start=True, stop=True,
                    )
                    if b_nonzero:
                        w = upool.tile([P, F_HID], f32, tag="w")
                        nc.vector.tensor_scalar(
                            w[:], pb[:], dis_sb[:, t : t + 1], None,
                            op0=mybir.AluOpType.mult,
                        )
                        nc.vector.tensor_tensor(
                            w[:], w[:], b1_sb[:], op=mybir.AluOpType.add
                        )
                        nc.scalar.activation(
                            ust[:, j, :], w[:],
                            mybir.ActivationFunctionType.Relu,
                            scale=dis_sb[:, t : t + 1],
                        )
                    else:
                        # u = dis * relu(dis*agg@W1) = relu(dis^2 * agg@W1)
                        nc.scalar.activation(
                            ust[:, j, :], pb[:],
                            mybir.ActivationFunctionType.Relu,
                            scale=dis2_sb[:, t : t + 1],
                        )
                q, scq = divmod(sci, NSCQ)
                nc.sync.dma_start(u2loc[q][scq], ust[:])
                if scq == NSCQ - 1:
                    nc.gpsimd.collective_compute(
                        "AllGather",
                        mybir.AluOpType.bypass,
                        replica_groups=[list(range(NCORES))],
                        ins=[u2loc[q][:]],
                        outs=[u2g[q][:]],
                    )

            agg_layer(1, l1_out)

            # ---------------- layer 2
            def l2_out(sci, ps):
                v = vpool.tile([P, SC], f16, tag="v")
                nc.scalar.copy(v[:], ps[:])
                yst = upool.tile([P, TPS, F_OUT], f32, tag="y")
                for j in range(TPS):
                    t = sci * TPS + j
                    pb = pmm.tile([P, F_OUT], f32, tag="pe")
                    nc.tensor.matmul(
                        pb[:], v[:, j * P : (j + 1) * P], W2_sb[:],
                        start=True, stop=True,
                    )
                    if b_nonzero:
                        y = upool.tile([P, F_OUT], f32, tag="yb")
                        nc.vector.tensor_scalar(
                            y[:], pb[:], dis_sb[:, t : t + 1], None,
                            op0=mybir.AluOpType.mult,
                        )
                        nc.vector.tensor_tensor(
                            yst[:, j, :], y[:], b2_sb[:],
                            op=mybir.AluOpType.add,
                        )
                    else:
                        nc.scalar.activation(
                            yst[:, j, :], pb[:],
                            mybir.ActivationFunctionType.Copy,
                            scale=dis_sb[:, t : t + 1],
                        )
                nc.sync.dma_start(out_d[sci], yst[:])

            agg_layer(2, l2_out)

    nc.compile()
    return nc


# ------------------------------------------------------------------ driver
_CACHE = {}
TRACE = False
LAST_RESULTS = None


def kernel(x, edge_index, W1, b1, W2, b2):
    from concourse.bass_utils import run_bass_kernel_spmd

    x = np.asarray(x)
    edge_index = np.asarray(edge_index)
    W1 = np.asarray(W1, dtype=np.float32)
    b1 = np.asarray(b1, dtype=np.float32)
    W2 = np.asarray(W2, dtype=np.float32)
    b2 = np.asarray(b2, dtype=np.float32)

    use_prep = os.environ.get("GCN2_PREP", "0") == "1"
    b_nonzero = bool(np.any(b1) or np.any(b2))
    pp = _prep(x, edge_index)
    key = (
        x.shape, edge_index.shape, pp["NG"], use_prep, b_nonzero,
        os.environ.get("GCN2_QN", "4"),
    )
    if key not in _CACHE:
        _CACHE[key] = _build(pp, use_prep, b_nonzero)
    nc = _CACHE[key]

    b1r = np.broadcast_to(b1, (P, F_HID)).copy()
    b2r = np.broadcast_to(b2, (P, F_OUT)).copy()
    in_maps = []
    for c in range(NCORES):
        m = {
            "xg": pp["xg"][c],
            "idxt": pp["idx_sb"][c],
            "srelt": pp["srel_sb"][c],
            "srel0t": pp["srel0_sb"][c],
            "dist": pp["dis_sb"][c],
            "dis2t": pp["dis2_sb"][c],
            "iotat": pp["iota_t"],
            "iotasc": pp["iota_sc"],
            "W1h": W1.astype(np.float16),
            "b1r": b1r,
            "W2h": W2.astype(np.float16),
            "b2r": b2r,
        }
        in_maps.append(m)
    res = run_bass_kernel_spmd(
        nc, in_maps, list(range(NCORES)), trace=TRACE
    )
    global LAST_RESULTS
    LAST_RESULTS = res
    outs = np.stack(
        [np.asarray(r["out"]).reshape(pp["R"], F_OUT) for r in res.results]
    )  # [C, R, FO]
    outs = outs.reshape(NCORES * pp["R"], F_OUT)
    return np.ascontiguousarray(outs[pp["node_row"]])
